# revision 4
# baseline (speedup 1.0000x reference)
"""Trainium2 Bass kernel for GQA decode-with-speculation attention.

Sharding: tensor-parallel over heads across 8 cores — core c owns kv head c
and query heads 4c..4c+3 (wq/wk/wv column-sharded, wo row-sharded, KV cache
sharded over kv heads). Each core computes a partial output [256, 4096]
(its heads' contribution through wo); the host sums the 8 partials.

Device-side layout strategy per core:
  - hiddenT [HID, TOK] fed from host, so projections need no on-device
    transpose of activations.
  - Q/K/V projections via f32r matmuls (full PE rate at N>=256), RoPE
    applied in token-major layout with sign-folded sin tables, then Q/K
    transposed via PE to [d, tok] for use as scores operands.
  - K cache fed host-pre-transposed as [B, D, T]; V cache natural [B, T, D].
  - Scores computed transposed: scT[t, (g,s)] = KT_chunk.T @ QT_b, exp on
    ScalarE straight out of PSUM (no max subtraction needed: |scaled
    scores| <~ 8, fp32 exp is exact enough), then PV uses E^T as the
    stationary operand against [V | 1] so one accumulating matmul chain
    yields both the numerator [64, 128] and the softmax denominator
    (column 128) with the partition-dim reduction done by the PE.
  - Normalize with per-partition reciprocal broadcast, PE-transpose to
    attnT [d, (g, tok)], then wo row-chunks in f32r.
"""

import math
import sys

sys.path.insert(0, "/opt/trn_rl_repo")

import numpy as np

import concourse.bass as bass
import concourse.tile as tile
from concourse import bacc, mybir
from concourse.bass_utils import run_bass_kernel_spmd

f32 = mybir.dt.float32
f32r = mybir.dt.float32r
EXP = mybir.ActivationFunctionType.Exp

B, S, T = 16, 16, 4096
H, HKV, D = 32, 8, 128
HID = H * D
G = H // HKV                 # 4 query heads per kv head
NCORES = 8
TOK = B * S                  # 256 tokens
DH = G * D                   # 512 per-core q/o dims
GS = G * S                   # 64 (g, s) rows per batch
NT = T // 128                # 32 prior key chunks
NH = HID // 128              # 32 hidden chunks
SCALE = 1.0 / math.sqrt(D)

_cache: dict = {}


def _build():
    nc = bacc.Bacc("TRN2", target_bir_lowering=False, debug=False,
                   num_devices=NCORES)

    def din(name, shape, dt):
        return nc.dram_tensor(name, shape, dt, kind="ExternalInput").ap()

    ht_d = din("ht", [HID, TOK], f32r)
    wq_d = din("wq", [HID, DH], f32r)
    wkv_d = din("wkv", [HID, 2 * D], f32r)
    wo_d = din("wo", [DH, HID], f32r)
    kt_d = din("kt", [B, D, T], f32)
    v_d = din("v", [B, T, D], f32)
    cosq_d = din("cosq", [TOK, DH], f32)
    sinq_d = din("sinq", [TOK, DH], f32)
    cosk_d = din("cosk", [TOK, D], f32)
    sink_d = din("sink", [TOK, D], f32)
    mask_d = din("mask", [S, GS], f32)
    ident_d = din("ident", [128, 128], f32)
    out_d = nc.dram_tensor("out", [TOK, HID], f32, kind="ExternalOutput").ap()

    with tile.TileContext(nc) as tc:
        with tc.tile_pool(name="const", bufs=1) as cp, \
             tc.tile_pool(name="persist", bufs=1) as pp:
            ident = cp.tile([128, 128], f32, tag="ident")
            nc.sync.dma_start(ident[:], ident_d[:])
            cosq = cp.tile([128, 2, DH], f32, tag="cosq")
            nc.sync.dma_start(cosq[:], cosq_d.rearrange("(c p) n -> p c n", p=128))
            sinq = cp.tile([128, 2, DH], f32, tag="sinq")
            nc.sync.dma_start(sinq[:], sinq_d.rearrange("(c p) n -> p c n", p=128))
            cosk = cp.tile([128, 2, D], f32, tag="cosk")
            nc.sync.dma_start(cosk[:], cosk_d.rearrange("(c p) n -> p c n", p=128))
            sink = cp.tile([128, 2, D], f32, tag="sink")
            nc.sync.dma_start(sink[:], sink_d.rearrange("(c p) n -> p c n", p=128))
            maskt = cp.tile([S, GS], f32, tag="mask")
            nc.sync.dma_start(maskt[:], mask_d[:])

            QT = pp.tile([128, G, TOK], f32, tag="QT")
            KTa = pp.tile([128, TOK], f32, tag="KTa")
            VaT = pp.tile([128, TOK], f32, tag="VaT")
            attnT = pp.tile([128, G, TOK], f32r, tag="attnT")

            # ---------------- projections + RoPE ----------------
            with tc.tile_pool(name="projw", bufs=1) as pwl, \
                 tc.tile_pool(name="projsb", bufs=2) as psb, \
                 tc.tile_pool(name="projps", bufs=2, space="PSUM") as pps, \
                 tc.tile_pool(name="trps", bufs=2, space="PSUM") as tps:
                ht = pwl.tile([128, NH, TOK], f32r, tag="ht")
                nc.sync.dma_start(ht[:], ht_d.rearrange("(c p) n -> p c n", p=128))
                wqs = pwl.tile([128, NH, DH], f32r, tag="wqs")
                nc.sync.dma_start(wqs[:], wq_d.rearrange("(c p) n -> p c n", p=128))
                wkvs = pwl.tile([128, NH, 2 * D], f32r, tag="wkvs")
                nc.sync.dma_start(wkvs[:], wkv_d.rearrange("(c p) n -> p c n", p=128))

                for t2 in range(2):
                    ts_ = slice(t2 * 128, (t2 + 1) * 128)
                    # Q = hidden @ wq  -> [tok, (g, d)]
                    pq = pps.tile([128, DH], f32, tag="pq")
                    for hh in range(NH):
                        nc.tensor.matmul(pq[:], ht[:, hh, ts_], wqs[:, hh, :],
                                         start=(hh == 0), stop=(hh == NH - 1))
                    qsb = psb.tile([128, DH], f32, tag="qsb")
                    nc.vector.tensor_copy(qsb[:], pq[:])
                    rot = psb.tile([128, DH], f32, tag="rot")
                    for g in range(G):
                        o = g * D
                        nc.vector.tensor_copy(rot[:, o:o + 64], qsb[:, o + 64:o + 128])
                        nc.vector.tensor_copy(rot[:, o + 64:o + 128], qsb[:, o:o + 64])
                    t1 = psb.tile([128, DH], f32, tag="t1")
                    nc.vector.tensor_mul(t1[:], qsb[:], cosq[:, t2, :])
                    rot2 = psb.tile([128, DH], f32, tag="rot2")
                    nc.vector.tensor_mul(rot2[:], rot[:], sinq[:, t2, :])
                    qr = psb.tile([128, DH], f32, tag="qr")
                    nc.vector.tensor_add(qr[:], t1[:], rot2[:])
                    for g in range(G):
                        tp = tps.tile([128, 128], f32, tag="tp")
                        nc.tensor.transpose(tp[:], qr[:, g * D:(g + 1) * D], ident[:])
                        nc.vector.tensor_copy(QT[:, g, ts_], tp[:])

                    # K|V = hidden @ [wk | wv]  -> [tok, 2*d]
                    pkv = pps.tile([128, 2 * D], f32, tag="pkv")
                    for hh in range(NH):
                        nc.tensor.matmul(pkv[:], ht[:, hh, ts_], wkvs[:, hh, :],
                                         start=(hh == 0), stop=(hh == NH - 1))
                    ksb = psb.tile([128, D], f32, tag="ksb")
                    nc.vector.tensor_copy(ksb[:], pkv[:, 0:D])
                    rotk = psb.tile([128, D], f32, tag="rotk")
                    nc.vector.tensor_copy(rotk[:, 0:64], ksb[:, 64:128])
                    nc.vector.tensor_copy(rotk[:, 64:128], ksb[:, 0:64])
                    t1k = psb.tile([128, D], f32, tag="t1k")
                    nc.vector.tensor_mul(t1k[:], ksb[:], cosk[:, t2, :])
                    rotk2 = psb.tile([128, D], f32, tag="rotk2")
                    nc.vector.tensor_mul(rotk2[:], rotk[:], sink[:, t2, :])
                    kr = psb.tile([128, D], f32, tag="kr")
                    nc.vector.tensor_add(kr[:], t1k[:], rotk2[:])
                    tpk = tps.tile([128, 128], f32, tag="tp")
                    nc.tensor.transpose(tpk[:], kr[:], ident[:])
                    nc.vector.tensor_copy(KTa[:, ts_], tpk[:])
                    vsb = psb.tile([128, D], f32, tag="vsb")
                    nc.vector.tensor_copy(vsb[:], pkv[:, D:2 * D])
                    tpv = tps.tile([128, 128], f32, tag="tp")
                    nc.tensor.transpose(tpv[:], vsb[:], ident[:])
                    nc.vector.tensor_copy(VaT[:, ts_], tpv[:])

            # ---------------- attention + output projection ----------------
            with tc.tile_pool(name="wosb", bufs=1) as wop, \
                 tc.tile_pool(name="ktp", bufs=2) as ktp, \
                 tc.tile_pool(name="vp", bufs=2) as vpl, \
                 tc.tile_pool(name="ep", bufs=4) as ep, \
                 tc.tile_pool(name="epi", bufs=2) as epi, \
                 tc.tile_pool(name="scps", bufs=3, space="PSUM") as scp, \
                 tc.tile_pool(name="numps", bufs=2, space="PSUM") as nump, \
                 tc.tile_pool(name="wops", bufs=2, space="PSUM") as wops:
                wos = wop.tile([128, G, HID], f32r, tag="wos")
                nc.sync.dma_start(wos[:], wo_d.rearrange("(g p) n -> p g n", p=128))

                def emit_wo(t2):
                    ts_ = slice(t2 * 128, (t2 + 1) * 128)
                    for nch in range(8):
                        ns_ = slice(nch * 512, (nch + 1) * 512)
                        pw = wops.tile([128, 512], f32, tag="pw")
                        for g in range(G):
                            nc.tensor.matmul(pw[:], attnT[:, g, ts_],
                                             wos[:, g, ns_],
                                             start=(g == 0), stop=(g == G - 1))
                        osb = epi.tile([128, 512], f32, tag="osb")
                        nc.vector.tensor_copy(osb[:], pw[:])
                        nc.sync.dma_start(out_d[ts_, ns_], osb[:])

                for b in range(B):
                    kt = ktp.tile([128, T], f32, tag="kt")
                    nc.sync.dma_start(kt[:], kt_d[b])
                    v = vpl.tile([128, NT, 132], f32, tag="v")
                    nc.sync.dma_start(v[:, :, 0:D],
                                      v_d[b].rearrange("(n p) d -> p n d", p=128))
                    nc.vector.memset(v[:, :, D:D + 1], 1.0)

                    qb = QT[:, :, b * S:(b + 1) * S]
                    num = nump.tile([GS, D + 1], f32, tag="num")
                    for n in range(NT):
                        sc = scp.tile([128, GS], f32, tag="sc")
                        nc.tensor.matmul(sc[:], kt[:, n * 128:(n + 1) * 128], qb,
                                         start=True, stop=True)
                        e = ep.tile([128, GS], f32, tag="e")
                        nc.scalar.activation(e[:], sc[:], EXP, scale=SCALE)
                        nc.tensor.matmul(num[:], e[:], v[:, n, 0:D + 1],
                                         start=(n == 0), stop=False)
                    # speculative (causal-masked) segment
                    sca = scp.tile([S, GS], f32, tag="sc")
                    nc.tensor.matmul(sca[:], KTa[:, b * S:(b + 1) * S], qb,
                                     start=True, stop=True)
                    nc.vector.tensor_add(sca[:], sca[:], maskt[:])
                    ea = ep.tile([S, GS], f32, tag="e")
                    nc.scalar.activation(ea[:], sca[:], EXP, scale=SCALE)
                    # bring this batch's active V rows back to partition base 0
                    tpv = scp.tile([S, D], f32, tag="sc")
                    nc.tensor.transpose(tpv[:], VaT[:, b * S:(b + 1) * S], ident[:])
                    vb = ep.tile([S, D + 4], f32, tag="vb")
                    nc.vector.tensor_copy(vb[:, 0:D], tpv[:])
                    nc.vector.memset(vb[:, D:D + 1], 1.0)
                    nc.tensor.matmul(num[:], ea[:], vb[:, 0:D + 1],
                                     start=False, stop=True)
                    # normalize + transpose into attnT
                    recip = epi.tile([GS, 1], f32, tag="recip")
                    nc.vector.reciprocal(recip[:], num[:, D:D + 1])
                    attn = epi.tile([GS, D], f32, tag="attn")
                    nc.vector.tensor_scalar_mul(attn[:], num[:, 0:D], recip[:])
                    tp2 = scp.tile([128, GS], f32, tag="sc")
                    nc.tensor.transpose(tp2[:], attn[:], ident[0:GS, 0:GS])
                    nc.vector.tensor_copy(
                        attnT[:, :, b * S:(b + 1) * S],
                        tp2[:].rearrange("p (g s) -> p g s", g=G))
                    if b == 7:
                        emit_wo(0)
                emit_wo(1)

    nc.compile()
    return nc


def _prep(x):
    hs = np.asarray(x["hidden_states"], np.float32)
    wq = np.asarray(x["wq"], np.float32)
    wk = np.asarray(x["wk"], np.float32)
    wv = np.asarray(x["wv"], np.float32)
    wo = np.asarray(x["wo"], np.float32)
    kp = np.asarray(x["k_prior"], np.float32)
    vp = np.asarray(x["v_prior"], np.float32)

    ht = np.ascontiguousarray(hs.reshape(TOK, HID).T)

    pos = np.asarray(x["position_ids"]).astype(np.float32).reshape(-1)  # [TOK]
    inv = (1.0 / (10000.0 ** (np.arange(0, D, 2, dtype=np.float32)
                              / np.float32(D)))).astype(np.float32)
    ang = pos[:, None] * inv[None, :]
    emb = np.concatenate([ang, ang], axis=1)
    cos = np.cos(emb).astype(np.float32)
    sin = np.sin(emb).astype(np.float32)
    sin2 = np.concatenate([-sin[:, :64], sin[:, 64:]], axis=1).astype(np.float32)
    cosq = np.ascontiguousarray(np.tile(cos, (1, G)))
    sinq = np.ascontiguousarray(np.tile(sin2, (1, G)))

    am = np.asarray(x["active_mask"])[0, 0]  # [S, S] bool, row=query s, col=key t
    mask = np.where(am.T, np.float32(0.0), np.float32(-1e9)).astype(np.float32)
    maskf = np.ascontiguousarray(np.tile(mask, (1, G)))
    ident = np.eye(128, dtype=np.float32)

    maps = []
    for c in range(NCORES):
        maps.append(dict(
            ht=ht,
            wq=np.ascontiguousarray(wq[:, c * DH:(c + 1) * DH]),
            wkv=np.ascontiguousarray(
                np.concatenate([wk[:, c * D:(c + 1) * D],
                                wv[:, c * D:(c + 1) * D]], axis=1)),
            wo=np.ascontiguousarray(wo[c * DH:(c + 1) * DH, :]),
            kt=np.ascontiguousarray(kp[:, c].transpose(0, 2, 1)),
            v=np.ascontiguousarray(vp[:, c]),
            cosq=cosq, sinq=sinq, cosk=cos, sink=sin2,
            mask=maskf, ident=ident,
        ))
    return maps


def kernel(**inputs) -> np.ndarray:
    if "nc" not in _cache:
        _cache["nc"] = _build()
    nc = _cache["nc"]
    in_maps = _prep(inputs)
    res = run_bass_kernel_spmd(nc, in_maps, list(range(NCORES)), **_cache.get("run_kwargs", {}))
    out = res.results[0]["out"].astype(np.float32).copy()
    for c in range(1, NCORES):
        out += res.results[c]["out"]
    if "last_result" in _cache or _cache.get("keep_result"):
        _cache["last_result"] = res
    return out.reshape(B, S, HID)


# revision 6
# speedup vs baseline: 2.1075x; 2.1075x over previous
"""Trainium2 Bass kernel for GQA decode-with-speculation attention.

Sharding: tensor-parallel over heads across 8 cores — core c owns kv head c
and query heads 4c..4c+3 (wq/wk/wv column-sharded, wo row-sharded, KV cache
sharded over kv heads). Each core computes a partial output [256, 4096]
(its heads' contribution through wo); the host sums the 8 partials.

Device-side strategy per core:
  - hiddenT [HID, TOK] fed from host, so projections need no on-device
    transpose of activations. Projections run in f32r (full PE rate at
    N>=256). RoPE is applied in token-major layout with sign-folded sin
    tables, then Q/K/V-active are PE-transposed to [d, tok].
  - K cache fed host-pre-transposed as [B, D, T] in bf16; V cache natural
    [B, T, D] in bf16. Attention matmuls run in bf16 (1 cyc/row vs 4 for
    fp32), accumulating in fp32 PSUM.
  - Scores computed transposed: scT[t, (g,s)] = KT_chunk.T @ QT_b into a
    shared [128, 512] PSUM tile (8 chunks per exp), exp on ScalarE
    straight out of PSUM (no max subtraction: |scaled scores| <~ 8, exp
    stays in fp32 range), then PV uses E^T as the stationary operand
    against [V | 1] so one accumulating matmul chain yields both the
    numerator [64, 128] and the softmax denominator (column 128) with the
    partition-dim reduction done by the PE.
  - Normalize with per-partition reciprocal broadcast, PE-transpose to
    attnT [d, (g, tok)], then wo row-chunks in f32r.
"""

import math
import sys

sys.path.insert(0, "/opt/trn_rl_repo")

import numpy as np
import ml_dtypes

import concourse.bass as bass
import concourse.tile as tile
from concourse import bacc, mybir
from concourse.bass_utils import run_bass_kernel_spmd

f32 = mybir.dt.float32
f32r = mybir.dt.float32r
bf16 = mybir.dt.bfloat16
EXP = mybir.ActivationFunctionType.Exp

B, S, T = 16, 16, 4096
H, HKV, D = 32, 8, 128
HID = H * D
G = H // HKV                 # 4 query heads per kv head
NCORES = 8
TOK = B * S                  # 256 tokens
DH = G * D                   # 512 per-core q/o dims
GS = G * S                   # 64 (g, s) rows per batch
NT = T // 128                # 32 prior key chunks
CG = 8                       # score chunks per exp group
NG = NT // CG                # 4 groups per batch
NH = HID // 128              # 32 hidden chunks
SCALE = 1.0 / math.sqrt(D)

_cache: dict = {}


def _build():
    nc = bacc.Bacc("TRN2", target_bir_lowering=False, debug=False,
                   num_devices=NCORES)

    def din(name, shape, dt):
        return nc.dram_tensor(name, shape, dt, kind="ExternalInput").ap()

    ht_d = din("ht", [HID, TOK], f32r)
    wq_d = din("wq", [HID, DH], f32r)
    wkv_d = din("wkv", [HID, 2 * D], f32r)
    wo_d = din("wo", [DH, HID], f32r)
    kt_d = din("kt", [B, D, T], bf16)
    v_d = din("v", [B, T, D], bf16)
    cosq_d = din("cosq", [TOK, DH], f32)
    sinq_d = din("sinq", [TOK, DH], f32)
    cosk_d = din("cosk", [TOK, D], f32)
    sink_d = din("sink", [TOK, D], f32)
    mask_d = din("mask", [S, GS], f32)
    ident_d = din("ident", [128, 128], f32)
    out_d = nc.dram_tensor("out", [TOK, HID], f32, kind="ExternalOutput").ap()

    with tile.TileContext(nc) as tc:
        with tc.tile_pool(name="const", bufs=1) as cp, \
             tc.tile_pool(name="persist", bufs=1) as pp, \
             tc.tile_pool(name="ktp", bufs=2) as ktp, \
             tc.tile_pool(name="vp", bufs=2) as vpl:

            # K/V prefetch for the first batches — no deps, DMA can start
            # while projection weights stream in.
            kv_tiles = {}

            def load_kv(b):
                kt = ktp.tile([128, T], bf16, tag="kt")
                nc.sync.dma_start(kt[:], kt_d[b])
                v = vpl.tile([128, NT, 132], bf16, tag="v")
                nc.sync.dma_start(v[:, :, 0:D],
                                  v_d[b].rearrange("(n p) d -> p n d", p=128))
                nc.vector.memset(v[:, :, D:D + 1], 1.0)
                kv_tiles[b] = (kt, v)

            for b in range(2):
                load_kv(b)

            ident = cp.tile([128, 128], f32, tag="ident")
            nc.sync.dma_start(ident[:], ident_d[:])
            cosq = cp.tile([128, 2, DH], f32, tag="cosq")
            nc.sync.dma_start(cosq[:], cosq_d.rearrange("(c p) n -> p c n", p=128))
            sinq = cp.tile([128, 2, DH], f32, tag="sinq")
            nc.sync.dma_start(sinq[:], sinq_d.rearrange("(c p) n -> p c n", p=128))
            cosk = cp.tile([128, 2, D], f32, tag="cosk")
            nc.sync.dma_start(cosk[:], cosk_d.rearrange("(c p) n -> p c n", p=128))
            sink = cp.tile([128, 2, D], f32, tag="sink")
            nc.sync.dma_start(sink[:], sink_d.rearrange("(c p) n -> p c n", p=128))
            maskt = cp.tile([S, GS], f32, tag="mask")
            nc.sync.dma_start(maskt[:], mask_d[:])

            QT = pp.tile([128, G, TOK], bf16, tag="QT")
            KTa = pp.tile([128, TOK], bf16, tag="KTa")
            VaT = pp.tile([128, TOK], f32, tag="VaT")
            attnT = pp.tile([128, G, TOK], f32r, tag="attnT")

            # ---------------- projections + RoPE ----------------
            with tc.tile_pool(name="projw", bufs=1) as pwl, \
                 tc.tile_pool(name="projsb", bufs=2) as psb, \
                 tc.tile_pool(name="projps", bufs=2, space="PSUM") as pps, \
                 tc.tile_pool(name="trps", bufs=2, space="PSUM") as tps:
                ht = pwl.tile([128, NH, TOK], f32r, tag="ht")
                wqs = pwl.tile([128, NH, DH], f32r, tag="wqs")
                wkvs = pwl.tile([128, NH, 2 * D], f32r, tag="wkvs")
                ht_r = ht_d.rearrange("(c p) n -> p c n", p=128)
                wq_r = wq_d.rearrange("(c p) n -> p c n", p=128)
                wkv_r = wkv_d.rearrange("(c p) n -> p c n", p=128)
                # per-chunk loads so matmuls can start on the first chunks
                for hh in range(NH):
                    nc.sync.dma_start(ht[:, hh, :], ht_r[:, hh, :])
                    nc.sync.dma_start(wqs[:, hh, :], wq_r[:, hh, :])
                    nc.sync.dma_start(wkvs[:, hh, :], wkv_r[:, hh, :])

                for t2 in range(2):
                    ts_ = slice(t2 * 128, (t2 + 1) * 128)
                    # Q = hidden @ wq  -> [tok, (g, d)]
                    pq = pps.tile([128, DH], f32, tag="pq")
                    for hh in range(NH):
                        nc.tensor.matmul(pq[:], ht[:, hh, ts_], wqs[:, hh, :],
                                         start=(hh == 0), stop=(hh == NH - 1))
                    qsb = psb.tile([128, DH], f32, tag="qsb")
                    nc.vector.tensor_copy(qsb[:], pq[:])
                    rot = psb.tile([128, DH], f32, tag="rot")
                    for g in range(G):
                        o = g * D
                        nc.vector.tensor_copy(rot[:, o:o + 64], qsb[:, o + 64:o + 128])
                        nc.vector.tensor_copy(rot[:, o + 64:o + 128], qsb[:, o:o + 64])
                    t1 = psb.tile([128, DH], f32, tag="t1")
                    nc.vector.tensor_mul(t1[:], qsb[:], cosq[:, t2, :])
                    rot2 = psb.tile([128, DH], f32, tag="rot2")
                    nc.vector.tensor_mul(rot2[:], rot[:], sinq[:, t2, :])
                    qr = psb.tile([128, DH], f32, tag="qr")
                    nc.vector.tensor_add(qr[:], t1[:], rot2[:])
                    for g in range(G):
                        tp = tps.tile([128, 128], f32, tag="tp")
                        nc.tensor.transpose(tp[:], qr[:, g * D:(g + 1) * D], ident[:])
                        nc.vector.tensor_copy(QT[:, g, ts_], tp[:])

                    # K|V = hidden @ [wk | wv]  -> [tok, 2*d]
                    pkv = pps.tile([128, 2 * D], f32, tag="pkv")
                    for hh in range(NH):
                        nc.tensor.matmul(pkv[:], ht[:, hh, ts_], wkvs[:, hh, :],
                                         start=(hh == 0), stop=(hh == NH - 1))
                    ksb = psb.tile([128, D], f32, tag="ksb")
                    nc.vector.tensor_copy(ksb[:], pkv[:, 0:D])
                    rotk = psb.tile([128, D], f32, tag="rotk")
                    nc.vector.tensor_copy(rotk[:, 0:64], ksb[:, 64:128])
                    nc.vector.tensor_copy(rotk[:, 64:128], ksb[:, 0:64])
                    t1k = psb.tile([128, D], f32, tag="t1k")
                    nc.vector.tensor_mul(t1k[:], ksb[:], cosk[:, t2, :])
                    rotk2 = psb.tile([128, D], f32, tag="rotk2")
                    nc.vector.tensor_mul(rotk2[:], rotk[:], sink[:, t2, :])
                    kr = psb.tile([128, D], f32, tag="kr")
                    nc.vector.tensor_add(kr[:], t1k[:], rotk2[:])
                    tpk = tps.tile([128, 128], f32, tag="tp")
                    nc.tensor.transpose(tpk[:], kr[:], ident[:])
                    nc.vector.tensor_copy(KTa[:, ts_], tpk[:])
                    vsb = psb.tile([128, D], f32, tag="vsb")
                    nc.vector.tensor_copy(vsb[:], pkv[:, D:2 * D])
                    tpv = tps.tile([128, 128], f32, tag="tp")
                    nc.tensor.transpose(tpv[:], vsb[:], ident[:])
                    nc.vector.tensor_copy(VaT[:, ts_], tpv[:])

            # ---------------- attention + output projection ----------------
            with tc.tile_pool(name="wosb", bufs=1) as wop, \
                 tc.tile_pool(name="ep", bufs=3) as ep, \
                 tc.tile_pool(name="epa", bufs=2) as epa, \
                 tc.tile_pool(name="epi", bufs=2) as epi, \
                 tc.tile_pool(name="scps", bufs=2, space="PSUM") as scp, \
                 tc.tile_pool(name="acps", bufs=2, space="PSUM") as acp, \
                 tc.tile_pool(name="numps", bufs=2, space="PSUM") as nump, \
                 tc.tile_pool(name="wops", bufs=2, space="PSUM") as wops:
                wos = wop.tile([128, G, HID], f32r, tag="wos")
                nc.sync.dma_start(wos[:], wo_d.rearrange("(g p) n -> p g n", p=128))

                def emit_wo(t2):
                    ts_ = slice(t2 * 128, (t2 + 1) * 128)
                    for nch in range(8):
                        ns_ = slice(nch * 512, (nch + 1) * 512)
                        pw = wops.tile([128, 512], f32, tag="pw")
                        for g in range(G):
                            nc.tensor.matmul(pw[:], attnT[:, g, ts_],
                                             wos[:, g, ns_],
                                             start=(g == 0), stop=(g == G - 1))
                        osb = epi.tile([128, 512], f32, tag="osb")
                        nc.vector.tensor_copy(osb[:], pw[:])
                        nc.sync.dma_start(out_d[ts_, ns_], osb[:])

                for b in range(B):
                    kt, v = kv_tiles.pop(b)
                    if b + 2 < B:
                        load_kv(b + 2)

                    qb = QT[:, :, b * S:(b + 1) * S]
                    num = nump.tile([GS, D + 1], f32, tag="num")
                    for gi in range(NG):
                        sc = scp.tile([128, CG, GS], f32, tag="sc")
                        for ci in range(CG):
                            n = gi * CG + ci
                            nc.tensor.matmul(sc[:, ci, :],
                                             kt[:, n * 128:(n + 1) * 128], qb,
                                             start=True, stop=True)
                        e = ep.tile([128, CG, GS], bf16, tag="e")
                        nc.scalar.activation(e[:], sc[:], EXP, scale=SCALE)
                        for ci in range(CG):
                            n = gi * CG + ci
                            nc.tensor.matmul(num[:], e[:, ci, :], v[:, n, 0:D + 1],
                                             start=(n == 0), stop=False)
                    # speculative (causal-masked) segment
                    sca = acp.tile([S, GS], f32, tag="a")
                    nc.tensor.matmul(sca[:], KTa[:, b * S:(b + 1) * S], qb,
                                     start=True, stop=True)
                    nc.vector.tensor_add(sca[:], sca[:], maskt[:])
                    ea = epa.tile([S, GS], bf16, tag="ea")
                    nc.scalar.activation(ea[:], sca[:], EXP, scale=SCALE)
                    # bring this batch's active V rows back to partition base 0
                    tpv = acp.tile([S, D], f32, tag="a")
                    nc.tensor.transpose(tpv[:], VaT[:, b * S:(b + 1) * S], ident[:])
                    vb = epa.tile([S, D + 4], bf16, tag="vb")
                    nc.vector.tensor_copy(vb[:, 0:D], tpv[:])
                    nc.vector.memset(vb[:, D:D + 1], 1.0)
                    nc.tensor.matmul(num[:], ea[:], vb[:, 0:D + 1],
                                     start=False, stop=True)
                    # normalize + transpose into attnT
                    recip = epi.tile([GS, 1], f32, tag="recip")
                    nc.vector.reciprocal(recip[:], num[:, D:D + 1])
                    attn = epi.tile([GS, D], f32, tag="attn")
                    nc.vector.tensor_scalar_mul(attn[:], num[:, 0:D], recip[:])
                    tp2 = acp.tile([128, GS], f32, tag="a")
                    nc.tensor.transpose(tp2[:], attn[:], ident[0:GS, 0:GS])
                    nc.vector.tensor_copy(
                        attnT[:, :, b * S:(b + 1) * S],
                        tp2[:].rearrange("p (g s) -> p g s", g=G))
                    if b == 7:
                        emit_wo(0)
                emit_wo(1)

    nc.compile()
    return nc


def _prep(x):
    hs = np.asarray(x["hidden_states"], np.float32)
    wq = np.asarray(x["wq"], np.float32)
    wk = np.asarray(x["wk"], np.float32)
    wv = np.asarray(x["wv"], np.float32)
    wo = np.asarray(x["wo"], np.float32)
    kp = np.asarray(x["k_prior"], np.float32)
    vp = np.asarray(x["v_prior"], np.float32)

    ht = np.ascontiguousarray(hs.reshape(TOK, HID).T)

    pos = np.asarray(x["position_ids"]).astype(np.float32).reshape(-1)  # [TOK]
    inv = (1.0 / (10000.0 ** (np.arange(0, D, 2, dtype=np.float32)
                              / np.float32(D)))).astype(np.float32)
    ang = pos[:, None] * inv[None, :]
    emb = np.concatenate([ang, ang], axis=1)
    cos = np.cos(emb).astype(np.float32)
    sin = np.sin(emb).astype(np.float32)
    sin2 = np.concatenate([-sin[:, :64], sin[:, 64:]], axis=1).astype(np.float32)
    cosq = np.ascontiguousarray(np.tile(cos, (1, G)))
    sinq = np.ascontiguousarray(np.tile(sin2, (1, G)))

    am = np.asarray(x["active_mask"])[0, 0]  # [S, S] bool, row=query s, col=key t
    mask = np.where(am.T, np.float32(0.0), np.float32(-1e9)).astype(np.float32)
    maskf = np.ascontiguousarray(np.tile(mask, (1, G)))
    ident = np.eye(128, dtype=np.float32)

    maps = []
    for c in range(NCORES):
        maps.append(dict(
            ht=ht,
            wq=np.ascontiguousarray(wq[:, c * DH:(c + 1) * DH]),
            wkv=np.ascontiguousarray(
                np.concatenate([wk[:, c * D:(c + 1) * D],
                                wv[:, c * D:(c + 1) * D]], axis=1)),
            wo=np.ascontiguousarray(wo[c * DH:(c + 1) * DH, :]),
            kt=np.ascontiguousarray(
                kp[:, c].transpose(0, 2, 1)).astype(ml_dtypes.bfloat16),
            v=np.ascontiguousarray(vp[:, c]).astype(ml_dtypes.bfloat16),
            cosq=cosq, sinq=sinq, cosk=cos, sink=sin2,
            mask=maskf, ident=ident,
        ))
    return maps


def kernel(**inputs) -> np.ndarray:
    if "nc" not in _cache:
        _cache["nc"] = _build()
    nc = _cache["nc"]
    in_maps = _prep(inputs)
    res = run_bass_kernel_spmd(nc, in_maps, list(range(NCORES)),
                               **_cache.get("run_kwargs", {}))
    out = res.results[0]["out"].astype(np.float32).copy()
    for c in range(1, NCORES):
        out += res.results[c]["out"]
    if _cache.get("keep_result"):
        _cache["last_result"] = res
    return out.reshape(B, S, HID)


# revision 7
# speedup vs baseline: 2.3087x; 1.0955x over previous
"""Trainium2 Bass kernel for GQA decode-with-speculation attention.

Sharding: tensor-parallel over heads across 8 cores — core c owns kv head c
and query heads 4c..4c+3 (wq/wk/wv column-sharded, wo row-sharded, KV cache
sharded over kv heads). Each core computes a partial output [256, 4096]
(its heads' contribution through wo); the host sums the 8 partials.

Device-side strategy per core:
  - hiddenT [HID, TOK] fed from host, so projections need no on-device
    transpose of activations. Projections run in f32r (full PE rate at
    N>=256). RoPE is applied in token-major layout with sign-folded sin
    tables, then Q/K/V-active are PE-transposed to [d, tok].
  - K cache fed host-pre-transposed as [B, D, T] in bf16; V cache natural
    [B, T, D] in bf16. Attention matmuls run in bf16 (1 cyc/row vs 4 for
    fp32), accumulating in fp32 PSUM.
  - Scores computed transposed: scT[t, (g,s)] = KT_chunk.T @ QT_b into a
    shared [128, 512] PSUM tile (8 chunks per exp), exp on ScalarE
    straight out of PSUM (no max subtraction: |scaled scores| <~ 8, exp
    stays in fp32 range), then PV uses E^T as the stationary operand
    against [V | 1] so one accumulating matmul chain yields both the
    numerator [64, 128] and the softmax denominator (column 128) with the
    partition-dim reduction done by the PE.
  - Normalize with per-partition reciprocal broadcast, PE-transpose to
    attnT [d, (g, tok)], then wo row-chunks in f32r.
"""

import math
import sys

sys.path.insert(0, "/opt/trn_rl_repo")

import numpy as np
import ml_dtypes

import concourse.bass as bass
import concourse.tile as tile
from concourse import bacc, mybir
from concourse.bass_utils import run_bass_kernel_spmd

f32 = mybir.dt.float32
f32r = mybir.dt.float32r
bf16 = mybir.dt.bfloat16
EXP = mybir.ActivationFunctionType.Exp

B, S, T = 16, 16, 4096
H, HKV, D = 32, 8, 128
HID = H * D
G = H // HKV                 # 4 query heads per kv head
NCORES = 8
TOK = B * S                  # 256 tokens
DH = G * D                   # 512 per-core q/o dims
GS = G * S                   # 64 (g, s) rows per batch
NT = T // 128                # 32 prior key chunks
CG = 8                       # score chunks per exp group
NG = NT // CG                # 4 groups per batch
NH = HID // 128              # 32 hidden chunks
SCALE = 1.0 / math.sqrt(D)

_cache: dict = {}


def _build():
    nc = bacc.Bacc("TRN2", target_bir_lowering=False, debug=False,
                   num_devices=NCORES)

    def din(name, shape, dt):
        return nc.dram_tensor(name, shape, dt, kind="ExternalInput").ap()

    ht_d = din("ht", [HID, TOK], bf16)
    wq_d = din("wq", [HID, DH], bf16)
    wkv_d = din("wkv", [HID, 2 * D], bf16)
    wo_d = din("wo", [DH, HID], bf16)
    kt_d = din("kt", [B, D, T], bf16)
    v_d = din("v", [B, 128, NT, D], bf16)
    cosq_d = din("cosq", [TOK, DH], f32)
    sinq_d = din("sinq", [TOK, DH], f32)
    cosk_d = din("cosk", [TOK, D], f32)
    sink_d = din("sink", [TOK, D], f32)
    mask_d = din("mask", [S, GS], f32)
    ident_d = din("ident", [128, 128], f32)
    out_d = nc.dram_tensor("out", [TOK, HID], f32, kind="ExternalOutput").ap()

    with tile.TileContext(nc) as tc:
        with tc.tile_pool(name="const", bufs=1) as cp, \
             tc.tile_pool(name="persist", bufs=1) as pp, \
             tc.tile_pool(name="ktp", bufs=2) as ktp, \
             tc.tile_pool(name="vp", bufs=2) as vpl:

            # K/V prefetch for the first batches — no deps, DMA can start
            # while projection weights stream in.
            kv_tiles = {}

            def load_kv(b):
                kt = ktp.tile([128, T], bf16, tag="kt")
                nc.scalar.dma_start(kt[:], kt_d[b])
                v = vpl.tile([128, NT, 132], bf16, tag="v")
                nc.scalar.dma_start(v[:, :, 0:D], v_d[b])
                nc.vector.memset(v[:, :, D:D + 1], 1.0)
                kv_tiles[b] = (kt, v)

            for b in range(2):
                load_kv(b)

            ident = cp.tile([128, 128], f32, tag="ident")
            nc.sync.dma_start(ident[:], ident_d[:])
            cosq = cp.tile([128, 2, DH], f32, tag="cosq")
            nc.sync.dma_start(cosq[:], cosq_d.rearrange("(c p) n -> p c n", p=128))
            sinq = cp.tile([128, 2, DH], f32, tag="sinq")
            nc.sync.dma_start(sinq[:], sinq_d.rearrange("(c p) n -> p c n", p=128))
            cosk = cp.tile([128, 2, D], f32, tag="cosk")
            nc.sync.dma_start(cosk[:], cosk_d.rearrange("(c p) n -> p c n", p=128))
            sink = cp.tile([128, 2, D], f32, tag="sink")
            nc.sync.dma_start(sink[:], sink_d.rearrange("(c p) n -> p c n", p=128))
            maskt = cp.tile([S, GS], f32, tag="mask")
            nc.sync.dma_start(maskt[:], mask_d[:])

            QT = pp.tile([128, G, TOK], bf16, tag="QT")
            KTa = pp.tile([128, TOK], bf16, tag="KTa")
            VaT = pp.tile([128, TOK], f32, tag="VaT")
            attnT = pp.tile([128, G, TOK], bf16, tag="attnT")

            # ---------------- projections + RoPE ----------------
            with tc.tile_pool(name="projw", bufs=1) as pwl, \
                 tc.tile_pool(name="projsb", bufs=2) as psb, \
                 tc.tile_pool(name="projps", bufs=2, space="PSUM") as pps, \
                 tc.tile_pool(name="trps", bufs=2, space="PSUM") as tps:
                ht = pwl.tile([128, NH, TOK], bf16, tag="ht")
                wqs = pwl.tile([128, NH, DH], bf16, tag="wqs")
                wkvs = pwl.tile([128, NH, 2 * D], bf16, tag="wkvs")
                ht_r = ht_d.rearrange("(c p) n -> p c n", p=128)
                wq_r = wq_d.rearrange("(c p) n -> p c n", p=128)
                wkv_r = wkv_d.rearrange("(c p) n -> p c n", p=128)
                # half-loads so matmuls can start after the first half
                hm = NH // 2
                for h2 in range(2):
                    hs_ = slice(h2 * hm, (h2 + 1) * hm)
                    nc.sync.dma_start(ht[:, hs_, :], ht_r[:, hs_, :])
                    nc.sync.dma_start(wqs[:, hs_, :], wq_r[:, hs_, :])
                    nc.sync.dma_start(wkvs[:, hs_, :], wkv_r[:, hs_, :])

                for t2 in range(2):
                    ts_ = slice(t2 * 128, (t2 + 1) * 128)
                    # Q = hidden @ wq  -> [tok, (g, d)]
                    pq = pps.tile([128, DH], f32, tag="pq")
                    for hh in range(NH):
                        nc.tensor.matmul(pq[:], ht[:, hh, ts_], wqs[:, hh, :],
                                         start=(hh == 0), stop=(hh == NH - 1))
                    qsb = psb.tile([128, DH], f32, tag="qsb")
                    nc.vector.tensor_copy(qsb[:], pq[:])
                    rot = psb.tile([128, DH], f32, tag="rot")
                    for g in range(G):
                        o = g * D
                        nc.vector.tensor_copy(rot[:, o:o + 64], qsb[:, o + 64:o + 128])
                        nc.vector.tensor_copy(rot[:, o + 64:o + 128], qsb[:, o:o + 64])
                    t1 = psb.tile([128, DH], f32, tag="t1")
                    nc.vector.tensor_mul(t1[:], qsb[:], cosq[:, t2, :])
                    rot2 = psb.tile([128, DH], f32, tag="rot2")
                    nc.vector.tensor_mul(rot2[:], rot[:], sinq[:, t2, :])
                    qr = psb.tile([128, DH], f32, tag="qr")
                    nc.vector.tensor_add(qr[:], t1[:], rot2[:])
                    for g in range(G):
                        tp = tps.tile([128, 128], f32, tag="tp")
                        nc.tensor.transpose(tp[:], qr[:, g * D:(g + 1) * D], ident[:])
                        nc.vector.tensor_copy(QT[:, g, ts_], tp[:])

                    # K|V = hidden @ [wk | wv]  -> [tok, 2*d]
                    pkv = pps.tile([128, 2 * D], f32, tag="pkv")
                    for hh in range(NH):
                        nc.tensor.matmul(pkv[:], ht[:, hh, ts_], wkvs[:, hh, :],
                                         start=(hh == 0), stop=(hh == NH - 1))
                    ksb = psb.tile([128, D], f32, tag="ksb")
                    nc.vector.tensor_copy(ksb[:], pkv[:, 0:D])
                    rotk = psb.tile([128, D], f32, tag="rotk")
                    nc.vector.tensor_copy(rotk[:, 0:64], ksb[:, 64:128])
                    nc.vector.tensor_copy(rotk[:, 64:128], ksb[:, 0:64])
                    t1k = psb.tile([128, D], f32, tag="t1k")
                    nc.vector.tensor_mul(t1k[:], ksb[:], cosk[:, t2, :])
                    rotk2 = psb.tile([128, D], f32, tag="rotk2")
                    nc.vector.tensor_mul(rotk2[:], rotk[:], sink[:, t2, :])
                    kr = psb.tile([128, D], f32, tag="kr")
                    nc.vector.tensor_add(kr[:], t1k[:], rotk2[:])
                    tpk = tps.tile([128, 128], f32, tag="tp")
                    nc.tensor.transpose(tpk[:], kr[:], ident[:])
                    nc.vector.tensor_copy(KTa[:, ts_], tpk[:])
                    vsb = psb.tile([128, D], f32, tag="vsb")
                    nc.vector.tensor_copy(vsb[:], pkv[:, D:2 * D])
                    tpv = tps.tile([128, 128], f32, tag="tp")
                    nc.tensor.transpose(tpv[:], vsb[:], ident[:])
                    nc.vector.tensor_copy(VaT[:, ts_], tpv[:])

            # ---------------- attention + output projection ----------------
            with tc.tile_pool(name="wosb", bufs=1) as wop, \
                 tc.tile_pool(name="ep", bufs=3) as ep, \
                 tc.tile_pool(name="epa", bufs=2) as epa, \
                 tc.tile_pool(name="epi", bufs=2) as epi, \
                 tc.tile_pool(name="scps", bufs=2, space="PSUM") as scp, \
                 tc.tile_pool(name="acps", bufs=2, space="PSUM") as acp, \
                 tc.tile_pool(name="numps", bufs=2, space="PSUM") as nump, \
                 tc.tile_pool(name="wops", bufs=2, space="PSUM") as wops:
                wos = wop.tile([128, G, HID], bf16, tag="wos")
                nc.sync.dma_start(wos[:], wo_d.rearrange("(g p) n -> p g n", p=128))

                def emit_wo(t2):
                    ts_ = slice(t2 * 128, (t2 + 1) * 128)
                    for nch in range(8):
                        ns_ = slice(nch * 512, (nch + 1) * 512)
                        pw = wops.tile([128, 512], f32, tag="pw")
                        for g in range(G):
                            nc.tensor.matmul(pw[:], attnT[:, g, ts_],
                                             wos[:, g, ns_],
                                             start=(g == 0), stop=(g == G - 1))
                        osb = epi.tile([128, 512], f32, tag="osb")
                        nc.vector.tensor_copy(osb[:], pw[:])
                        nc.sync.dma_start(out_d[ts_, ns_], osb[:])

                for b in range(B):
                    kt, v = kv_tiles.pop(b)
                    if b + 2 < B:
                        load_kv(b + 2)

                    qb = QT[:, :, b * S:(b + 1) * S]
                    num = nump.tile([GS, D + 1], f32, tag="num")
                    for gi in range(NG):
                        sc = scp.tile([128, CG, GS], f32, tag="sc")
                        for ci in range(CG):
                            n = gi * CG + ci
                            nc.tensor.matmul(sc[:, ci, :],
                                             kt[:, n * 128:(n + 1) * 128], qb,
                                             start=True, stop=True)
                        e = ep.tile([128, CG, GS], bf16, tag="e")
                        nc.scalar.activation(e[:], sc[:], EXP, scale=SCALE)
                        for ci in range(CG):
                            n = gi * CG + ci
                            nc.tensor.matmul(num[:], e[:, ci, :], v[:, n, 0:D + 1],
                                             start=(n == 0), stop=False)
                    # speculative (causal-masked) segment
                    sca = acp.tile([S, GS], f32, tag="a")
                    nc.tensor.matmul(sca[:], KTa[:, b * S:(b + 1) * S], qb,
                                     start=True, stop=True)
                    nc.vector.tensor_add(sca[:], sca[:], maskt[:])
                    ea = epa.tile([S, GS], bf16, tag="ea")
                    nc.scalar.activation(ea[:], sca[:], EXP, scale=SCALE)
                    # bring this batch's active V rows back to partition base 0
                    tpv = acp.tile([S, D], f32, tag="a")
                    nc.tensor.transpose(tpv[:], VaT[:, b * S:(b + 1) * S], ident[:])
                    vb = epa.tile([S, D + 4], bf16, tag="vb")
                    nc.vector.tensor_copy(vb[:, 0:D], tpv[:])
                    nc.vector.memset(vb[:, D:D + 1], 1.0)
                    nc.tensor.matmul(num[:], ea[:], vb[:, 0:D + 1],
                                     start=False, stop=True)
                    # normalize + transpose into attnT
                    recip = epi.tile([GS, 1], f32, tag="recip")
                    nc.vector.reciprocal(recip[:], num[:, D:D + 1])
                    attn = epi.tile([GS, D], f32, tag="attn")
                    nc.vector.tensor_scalar_mul(attn[:], num[:, 0:D], recip[:])
                    tp2 = acp.tile([128, GS], f32, tag="a")
                    nc.tensor.transpose(tp2[:], attn[:], ident[0:GS, 0:GS])
                    nc.vector.tensor_copy(
                        attnT[:, :, b * S:(b + 1) * S],
                        tp2[:].rearrange("p (g s) -> p g s", g=G))
                    if b == 7:
                        emit_wo(0)
                emit_wo(1)

    nc.compile()
    return nc


def _prep(x):
    hs = np.asarray(x["hidden_states"], np.float32)
    wq = np.asarray(x["wq"], np.float32)
    wk = np.asarray(x["wk"], np.float32)
    wv = np.asarray(x["wv"], np.float32)
    wo = np.asarray(x["wo"], np.float32)
    kp = np.asarray(x["k_prior"], np.float32)
    vp = np.asarray(x["v_prior"], np.float32)

    ht = np.ascontiguousarray(hs.reshape(TOK, HID).T)

    pos = np.asarray(x["position_ids"]).astype(np.float32).reshape(-1)  # [TOK]
    inv = (1.0 / (10000.0 ** (np.arange(0, D, 2, dtype=np.float32)
                              / np.float32(D)))).astype(np.float32)
    ang = pos[:, None] * inv[None, :]
    emb = np.concatenate([ang, ang], axis=1)
    cos = np.cos(emb).astype(np.float32)
    sin = np.sin(emb).astype(np.float32)
    sin2 = np.concatenate([-sin[:, :64], sin[:, 64:]], axis=1).astype(np.float32)
    cosq = np.ascontiguousarray(np.tile(cos, (1, G)))
    sinq = np.ascontiguousarray(np.tile(sin2, (1, G)))

    am = np.asarray(x["active_mask"])[0, 0]  # [S, S] bool, row=query s, col=key t
    mask = np.where(am.T, np.float32(0.0), np.float32(-1e9)).astype(np.float32)
    maskf = np.ascontiguousarray(np.tile(mask, (1, G)))
    ident = np.eye(128, dtype=np.float32)

    maps = []
    for c in range(NCORES):
        maps.append(dict(
            ht=ht.astype(ml_dtypes.bfloat16),
            wq=wq[:, c * DH:(c + 1) * DH].astype(ml_dtypes.bfloat16),
            wkv=np.concatenate([wk[:, c * D:(c + 1) * D],
                                wv[:, c * D:(c + 1) * D]],
                               axis=1).astype(ml_dtypes.bfloat16),
            wo=wo[c * DH:(c + 1) * DH, :].astype(ml_dtypes.bfloat16),
            kt=np.ascontiguousarray(
                kp[:, c].transpose(0, 2, 1)).astype(ml_dtypes.bfloat16),
            v=np.ascontiguousarray(
                vp[:, c].reshape(B, NT, 128, D).transpose(0, 2, 1, 3)
            ).astype(ml_dtypes.bfloat16),
            cosq=cosq, sinq=sinq, cosk=cos, sink=sin2,
            mask=maskf, ident=ident,
        ))
    return maps


def kernel(**inputs) -> np.ndarray:
    if "nc" not in _cache:
        _cache["nc"] = _build()
    nc = _cache["nc"]
    in_maps = _prep(inputs)
    res = run_bass_kernel_spmd(nc, in_maps, list(range(NCORES)),
                               **_cache.get("run_kwargs", {}))
    out = res.results[0]["out"].astype(np.float32).copy()
    for c in range(1, NCORES):
        out += res.results[c]["out"]
    if _cache.get("keep_result"):
        _cache["last_result"] = res
    return out.reshape(B, S, HID)


# revision 8
# speedup vs baseline: 2.6724x; 1.1575x over previous
"""Trainium2 Bass kernel for GQA decode-with-speculation attention.

Sharding: tensor-parallel over heads across 8 cores — core c owns kv head c
and query heads 4c..4c+3 (wq/wk/wv column-sharded, wo row-sharded, KV cache
sharded over kv heads). Each core computes a partial output [256, 4096]
(its heads' contribution through wo); the host sums the 8 partials.

Device-side strategy per core:
  - hiddenT [HID, TOK] fed from host, so projections need no on-device
    transpose of activations. Projections run in f32r (full PE rate at
    N>=256). RoPE is applied in token-major layout with sign-folded sin
    tables, then Q/K/V-active are PE-transposed to [d, tok].
  - K cache fed host-pre-transposed as [B, D, T] in bf16; V cache natural
    [B, T, D] in bf16. Attention matmuls run in bf16 (1 cyc/row vs 4 for
    fp32), accumulating in fp32 PSUM.
  - Scores computed transposed: scT[t, (g,s)] = KT_chunk.T @ QT_b into a
    shared [128, 512] PSUM tile (8 chunks per exp), exp on ScalarE
    straight out of PSUM (no max subtraction: |scaled scores| <~ 8, exp
    stays in fp32 range), then PV uses E^T as the stationary operand
    against [V | 1] so one accumulating matmul chain yields both the
    numerator [64, 128] and the softmax denominator (column 128) with the
    partition-dim reduction done by the PE.
  - Normalize with per-partition reciprocal broadcast, PE-transpose to
    attnT [d, (g, tok)], then wo row-chunks in f32r.
"""

import math
import sys

sys.path.insert(0, "/opt/trn_rl_repo")

import numpy as np
import ml_dtypes

import concourse.bass as bass
import concourse.tile as tile
from concourse import bacc, mybir
from concourse.bass_utils import run_bass_kernel_spmd

f32 = mybir.dt.float32
f32r = mybir.dt.float32r
bf16 = mybir.dt.bfloat16
EXP = mybir.ActivationFunctionType.Exp

B, S, T = 16, 16, 4096
H, HKV, D = 32, 8, 128
HID = H * D
G = H // HKV                 # 4 query heads per kv head
NCORES = 8
TOK = B * S                  # 256 tokens
DH = G * D                   # 512 per-core q/o dims
GS = G * S                   # 64 (g, s) rows per batch
NT = T // 128                # 32 prior key chunks
CG = 8                       # score chunks per exp group
NG = NT // CG                # 4 groups per batch
NH = HID // 128              # 32 hidden chunks
SCALE = 1.0 / math.sqrt(D)

_cache: dict = {}


def _build():
    nc = bacc.Bacc("TRN2", target_bir_lowering=False, debug=False,
                   num_devices=NCORES)

    def din(name, shape, dt):
        return nc.dram_tensor(name, shape, dt, kind="ExternalInput").ap()

    ht_d = din("ht", [HID, TOK], bf16)
    wq_d = din("wq", [HID, DH], bf16)
    wkv_d = din("wkv", [HID, 2 * D], bf16)
    wo_d = din("wo", [DH, HID], bf16)
    kt_d = din("kt", [B, D, T], bf16)
    v_d = din("v", [B, 128, NT, 132], bf16)
    cosq_d = din("cosq", [TOK, DH], f32)
    sinq_d = din("sinq", [TOK, DH], f32)
    cosk_d = din("cosk", [TOK, D], f32)
    sink_d = din("sink", [TOK, D], f32)
    mask_d = din("mask", [S, GS], f32)
    ident_d = din("ident", [128, 128], f32)
    out_d = nc.dram_tensor("out", [TOK, HID], f32, kind="ExternalOutput").ap()

    with tile.TileContext(nc) as tc:
        with tc.tile_pool(name="const", bufs=1) as cp, \
             tc.tile_pool(name="persist", bufs=1) as pp, \
             tc.tile_pool(name="ktp", bufs=2) as ktp, \
             tc.tile_pool(name="vp", bufs=2) as vpl:

            # K/V prefetch for the first batches — no deps, DMA can start
            # while projection weights stream in.
            kv_tiles = {}

            def load_kv(b):
                kt = ktp.tile([128, T], bf16, tag="kt")
                nc.scalar.dma_start(kt[:], kt_d[b])
                v = vpl.tile([128, NT, 132], bf16, tag="v")
                nc.sync.dma_start(v[:], v_d[b])
                kv_tiles[b] = (kt, v)

            for b in range(2):
                load_kv(b)

            ident = cp.tile([128, 128], f32, tag="ident")
            nc.sync.dma_start(ident[:], ident_d[:])
            cosq = cp.tile([128, 2, DH], f32, tag="cosq")
            nc.sync.dma_start(cosq[:], cosq_d.rearrange("(c p) n -> p c n", p=128))
            sinq = cp.tile([128, 2, DH], f32, tag="sinq")
            nc.sync.dma_start(sinq[:], sinq_d.rearrange("(c p) n -> p c n", p=128))
            cosk = cp.tile([128, 2, D], f32, tag="cosk")
            nc.sync.dma_start(cosk[:], cosk_d.rearrange("(c p) n -> p c n", p=128))
            sink = cp.tile([128, 2, D], f32, tag="sink")
            nc.sync.dma_start(sink[:], sink_d.rearrange("(c p) n -> p c n", p=128))
            maskt = cp.tile([S, GS], f32, tag="mask")
            nc.sync.dma_start(maskt[:], mask_d[:])

            QT = pp.tile([128, G, TOK], bf16, tag="QT")
            KTa = pp.tile([128, TOK], bf16, tag="KTa")
            VaT = pp.tile([128, TOK], f32, tag="VaT")
            attnT = pp.tile([128, G, TOK], bf16, tag="attnT")

            # ---------------- projections + RoPE ----------------
            with tc.tile_pool(name="projw", bufs=1) as pwl, \
                 tc.tile_pool(name="projsb", bufs=2) as psb, \
                 tc.tile_pool(name="projps", bufs=2, space="PSUM") as pps, \
                 tc.tile_pool(name="trps", bufs=2, space="PSUM") as tps:
                ht = pwl.tile([128, NH, TOK], bf16, tag="ht")
                wqs = pwl.tile([128, NH, DH], bf16, tag="wqs")
                wkvs = pwl.tile([128, NH, 2 * D], bf16, tag="wkvs")
                ht_r = ht_d.rearrange("(c p) n -> p c n", p=128)
                wq_r = wq_d.rearrange("(c p) n -> p c n", p=128)
                wkv_r = wkv_d.rearrange("(c p) n -> p c n", p=128)
                # half-loads so matmuls can start after the first half
                hm = NH // 2
                for h2 in range(2):
                    hs_ = slice(h2 * hm, (h2 + 1) * hm)
                    nc.sync.dma_start(ht[:, hs_, :], ht_r[:, hs_, :])
                    nc.sync.dma_start(wqs[:, hs_, :], wq_r[:, hs_, :])
                    nc.sync.dma_start(wkvs[:, hs_, :], wkv_r[:, hs_, :])

                for t2 in range(2):
                    ts_ = slice(t2 * 128, (t2 + 1) * 128)
                    # Q = hidden @ wq  -> [tok, (g, d)]
                    pq = pps.tile([128, DH], f32, tag="pq")
                    for hh in range(NH):
                        nc.tensor.matmul(pq[:], ht[:, hh, ts_], wqs[:, hh, :],
                                         start=(hh == 0), stop=(hh == NH - 1))
                    qsb = psb.tile([128, DH], f32, tag="qsb")
                    nc.vector.tensor_copy(qsb[:], pq[:])
                    rot = psb.tile([128, DH], f32, tag="rot")
                    for g in range(G):
                        o = g * D
                        nc.vector.tensor_copy(rot[:, o:o + 64], qsb[:, o + 64:o + 128])
                        nc.vector.tensor_copy(rot[:, o + 64:o + 128], qsb[:, o:o + 64])
                    t1 = psb.tile([128, DH], f32, tag="t1")
                    nc.vector.tensor_mul(t1[:], qsb[:], cosq[:, t2, :])
                    rot2 = psb.tile([128, DH], f32, tag="rot2")
                    nc.vector.tensor_mul(rot2[:], rot[:], sinq[:, t2, :])
                    qr = psb.tile([128, DH], f32, tag="qr")
                    nc.vector.tensor_add(qr[:], t1[:], rot2[:])
                    for g in range(G):
                        tp = tps.tile([128, 128], f32, tag="tp")
                        nc.tensor.transpose(tp[:], qr[:, g * D:(g + 1) * D], ident[:])
                        nc.vector.tensor_copy(QT[:, g, ts_], tp[:])

                    # K|V = hidden @ [wk | wv]  -> [tok, 2*d]
                    pkv = pps.tile([128, 2 * D], f32, tag="pkv")
                    for hh in range(NH):
                        nc.tensor.matmul(pkv[:], ht[:, hh, ts_], wkvs[:, hh, :],
                                         start=(hh == 0), stop=(hh == NH - 1))
                    ksb = psb.tile([128, D], f32, tag="ksb")
                    nc.vector.tensor_copy(ksb[:], pkv[:, 0:D])
                    rotk = psb.tile([128, D], f32, tag="rotk")
                    nc.vector.tensor_copy(rotk[:, 0:64], ksb[:, 64:128])
                    nc.vector.tensor_copy(rotk[:, 64:128], ksb[:, 0:64])
                    t1k = psb.tile([128, D], f32, tag="t1k")
                    nc.vector.tensor_mul(t1k[:], ksb[:], cosk[:, t2, :])
                    rotk2 = psb.tile([128, D], f32, tag="rotk2")
                    nc.vector.tensor_mul(rotk2[:], rotk[:], sink[:, t2, :])
                    kr = psb.tile([128, D], f32, tag="kr")
                    nc.vector.tensor_add(kr[:], t1k[:], rotk2[:])
                    tpk = tps.tile([128, 128], f32, tag="tp")
                    nc.tensor.transpose(tpk[:], kr[:], ident[:])
                    nc.vector.tensor_copy(KTa[:, ts_], tpk[:])
                    vsb = psb.tile([128, D], f32, tag="vsb")
                    nc.vector.tensor_copy(vsb[:], pkv[:, D:2 * D])
                    tpv = tps.tile([128, 128], f32, tag="tp")
                    nc.tensor.transpose(tpv[:], vsb[:], ident[:])
                    nc.vector.tensor_copy(VaT[:, ts_], tpv[:])

            # ---------------- attention + output projection ----------------
            with tc.tile_pool(name="wosb", bufs=1) as wop, \
                 tc.tile_pool(name="ep", bufs=3) as ep, \
                 tc.tile_pool(name="epa", bufs=2) as epa, \
                 tc.tile_pool(name="epi", bufs=2) as epi, \
                 tc.tile_pool(name="scps", bufs=2, space="PSUM") as scp, \
                 tc.tile_pool(name="acps", bufs=2, space="PSUM") as acp, \
                 tc.tile_pool(name="numps", bufs=2, space="PSUM") as nump, \
                 tc.tile_pool(name="wops", bufs=2, space="PSUM") as wops:
                wos = wop.tile([128, G, HID], bf16, tag="wos")
                nc.sync.dma_start(wos[:], wo_d.rearrange("(g p) n -> p g n", p=128))

                def emit_wo(t2):
                    ts_ = slice(t2 * 128, (t2 + 1) * 128)
                    osb = epi.tile([128, HID], f32, tag="osb")
                    for nch in range(8):
                        ns_ = slice(nch * 512, (nch + 1) * 512)
                        pw = wops.tile([128, 512], f32, tag="pw")
                        for g in range(G):
                            nc.tensor.matmul(pw[:], attnT[:, g, ts_],
                                             wos[:, g, ns_],
                                             start=(g == 0), stop=(g == G - 1))
                        nc.vector.tensor_copy(osb[:, ns_], pw[:])
                    nc.sync.dma_start(out_d[ts_, :], osb[:])

                for b in range(B):
                    kt, v = kv_tiles.pop(b)
                    if b + 2 < B:
                        load_kv(b + 2)

                    qb = QT[:, :, b * S:(b + 1) * S]
                    num = nump.tile([GS, D + 1], f32, tag="num")
                    for gi in range(NG):
                        sc = scp.tile([128, CG, GS], f32, tag="sc")
                        for ci in range(CG):
                            n = gi * CG + ci
                            nc.tensor.matmul(sc[:, ci, :],
                                             kt[:, n * 128:(n + 1) * 128], qb,
                                             start=True, stop=True)
                        e = ep.tile([128, CG, GS], bf16, tag="e")
                        nc.scalar.activation(e[:], sc[:], EXP, scale=SCALE)
                        for ci in range(CG):
                            n = gi * CG + ci
                            nc.tensor.matmul(num[:], e[:, ci, :], v[:, n, 0:D + 1],
                                             start=(n == 0), stop=False)
                    # speculative (causal-masked) segment
                    sca = acp.tile([S, GS], f32, tag="a")
                    nc.tensor.matmul(sca[:], KTa[:, b * S:(b + 1) * S], qb,
                                     start=True, stop=True)
                    nc.vector.tensor_add(sca[:], sca[:], maskt[:])
                    ea = epa.tile([S, GS], bf16, tag="ea")
                    nc.scalar.activation(ea[:], sca[:], EXP, scale=SCALE)
                    # bring this batch's active V rows back to partition base 0
                    tpv = acp.tile([S, D], f32, tag="a")
                    nc.tensor.transpose(tpv[:], VaT[:, b * S:(b + 1) * S], ident[:])
                    vb = epa.tile([S, D + 4], bf16, tag="vb")
                    nc.vector.tensor_copy(vb[:, 0:D], tpv[:])
                    nc.vector.memset(vb[:, D:D + 1], 1.0)
                    nc.tensor.matmul(num[:], ea[:], vb[:, 0:D + 1],
                                     start=False, stop=True)
                    # normalize + transpose into attnT
                    recip = epi.tile([GS, 1], f32, tag="recip")
                    nc.vector.reciprocal(recip[:], num[:, D:D + 1])
                    attn = epi.tile([GS, D], f32, tag="attn")
                    nc.vector.tensor_scalar_mul(attn[:], num[:, 0:D], recip[:])
                    tp2 = acp.tile([128, GS], f32, tag="a")
                    nc.tensor.transpose(tp2[:], attn[:], ident[0:GS, 0:GS])
                    nc.vector.tensor_copy(
                        attnT[:, :, b * S:(b + 1) * S],
                        tp2[:].rearrange("p (g s) -> p g s", g=G))
                    if b == 7:
                        emit_wo(0)
                emit_wo(1)

    nc.compile()
    return nc


def _vpad(vc):
    # [T, D] x B -> [B, 128, NT, 132] with a ones column at 128, zeros after
    v4 = vc.reshape(B, NT, 128, D).transpose(0, 2, 1, 3)
    out = np.zeros((B, 128, NT, 132), np.float32)
    out[..., :D] = v4
    out[..., D] = 1.0
    return out.astype(ml_dtypes.bfloat16)


def _prep(x):
    hs = np.asarray(x["hidden_states"], np.float32)
    wq = np.asarray(x["wq"], np.float32)
    wk = np.asarray(x["wk"], np.float32)
    wv = np.asarray(x["wv"], np.float32)
    wo = np.asarray(x["wo"], np.float32)
    kp = np.asarray(x["k_prior"], np.float32)
    vp = np.asarray(x["v_prior"], np.float32)

    ht = np.ascontiguousarray(hs.reshape(TOK, HID).T)

    pos = np.asarray(x["position_ids"]).astype(np.float32).reshape(-1)  # [TOK]
    inv = (1.0 / (10000.0 ** (np.arange(0, D, 2, dtype=np.float32)
                              / np.float32(D)))).astype(np.float32)
    ang = pos[:, None] * inv[None, :]
    emb = np.concatenate([ang, ang], axis=1)
    cos = np.cos(emb).astype(np.float32)
    sin = np.sin(emb).astype(np.float32)
    sin2 = np.concatenate([-sin[:, :64], sin[:, 64:]], axis=1).astype(np.float32)
    cosq = np.ascontiguousarray(np.tile(cos, (1, G)))
    sinq = np.ascontiguousarray(np.tile(sin2, (1, G)))

    am = np.asarray(x["active_mask"])[0, 0]  # [S, S] bool, row=query s, col=key t
    mask = np.where(am.T, np.float32(0.0), np.float32(-1e9)).astype(np.float32)
    maskf = np.ascontiguousarray(np.tile(mask, (1, G)))
    ident = np.eye(128, dtype=np.float32)

    maps = []
    for c in range(NCORES):
        maps.append(dict(
            ht=ht.astype(ml_dtypes.bfloat16),
            wq=wq[:, c * DH:(c + 1) * DH].astype(ml_dtypes.bfloat16),
            wkv=np.concatenate([wk[:, c * D:(c + 1) * D],
                                wv[:, c * D:(c + 1) * D]],
                               axis=1).astype(ml_dtypes.bfloat16),
            wo=wo[c * DH:(c + 1) * DH, :].astype(ml_dtypes.bfloat16),
            kt=np.ascontiguousarray(
                kp[:, c].transpose(0, 2, 1)).astype(ml_dtypes.bfloat16),
            v=_vpad(vp[:, c]),
            cosq=cosq, sinq=sinq, cosk=cos, sink=sin2,
            mask=maskf, ident=ident,
        ))
    return maps


def kernel(**inputs) -> np.ndarray:
    if "nc" not in _cache:
        _cache["nc"] = _build()
    nc = _cache["nc"]
    in_maps = _prep(inputs)
    res = run_bass_kernel_spmd(nc, in_maps, list(range(NCORES)),
                               **_cache.get("run_kwargs", {}))
    out = res.results[0]["out"].astype(np.float32).copy()
    for c in range(1, NCORES):
        out += res.results[c]["out"]
    if _cache.get("keep_result"):
        _cache["last_result"] = res
    return out.reshape(B, S, HID)


# revision 9
# speedup vs baseline: 2.8448x; 1.0645x over previous
"""Trainium2 Bass kernel for GQA decode-with-speculation attention.

Sharding: tensor-parallel over heads across 8 cores — core c owns kv head c
and query heads 4c..4c+3 (wq/wk/wv column-sharded, wo row-sharded, KV cache
sharded over kv heads). Each core computes a partial output [256, 4096]
(its heads' contribution through wo); the host sums the 8 partials.

Device-side strategy per core:
  - hiddenT [HID, TOK] fed from host, so projections need no on-device
    transpose of activations. Projections run in f32r (full PE rate at
    N>=256). RoPE is applied in token-major layout with sign-folded sin
    tables, then Q/K/V-active are PE-transposed to [d, tok].
  - K cache fed host-pre-transposed as [B, D, T] in bf16; V cache natural
    [B, T, D] in bf16. Attention matmuls run in bf16 (1 cyc/row vs 4 for
    fp32), accumulating in fp32 PSUM.
  - Scores computed transposed: scT[t, (g,s)] = KT_chunk.T @ QT_b into a
    shared [128, 512] PSUM tile (8 chunks per exp), exp on ScalarE
    straight out of PSUM (no max subtraction: |scaled scores| <~ 8, exp
    stays in fp32 range), then PV uses E^T as the stationary operand
    against [V | 1] so one accumulating matmul chain yields both the
    numerator [64, 128] and the softmax denominator (column 128) with the
    partition-dim reduction done by the PE.
  - Normalize with per-partition reciprocal broadcast, PE-transpose to
    attnT [d, (g, tok)], then wo row-chunks in f32r.
"""

import math
import sys

sys.path.insert(0, "/opt/trn_rl_repo")

import numpy as np
import ml_dtypes

import concourse.bass as bass
import concourse.tile as tile
from concourse import bacc, mybir
from concourse.bass_utils import run_bass_kernel_spmd

f32 = mybir.dt.float32
f32r = mybir.dt.float32r
bf16 = mybir.dt.bfloat16
EXP = mybir.ActivationFunctionType.Exp

B, S, T = 16, 16, 4096
H, HKV, D = 32, 8, 128
HID = H * D
G = H // HKV                 # 4 query heads per kv head
NCORES = 8
TOK = B * S                  # 256 tokens
DH = G * D                   # 512 per-core q/o dims
GS = G * S                   # 64 (g, s) rows per batch
NT = T // 128                # 32 prior key chunks
CG = 8                       # score chunks per exp group
NG = NT // CG                # 4 groups per batch
NH = HID // 128              # 32 hidden chunks
SCALE = 1.0 / math.sqrt(D)

_cache: dict = {}


def _build():
    nc = bacc.Bacc("TRN2", target_bir_lowering=False, debug=False,
                   num_devices=NCORES)

    def din(name, shape, dt):
        return nc.dram_tensor(name, shape, dt, kind="ExternalInput").ap()

    ht_d = din("ht", [128, NH, TOK], bf16)
    wq_d = din("wq", [128, NH, DH], bf16)
    wkv_d = din("wkv", [128, NH, 2 * D], bf16)
    wo_d = din("wo", [128, G, HID], bf16)
    kt_d = din("kt", [B, D, T], bf16)
    v_d = din("v", [B, 128, NT, 132], bf16)
    cosq_d = din("cosq", [128, 2, DH], f32)
    sinq_d = din("sinq", [128, 2, DH], f32)
    cosk_d = din("cosk", [128, 2, D], f32)
    sink_d = din("sink", [128, 2, D], f32)
    mask_d = din("mask", [S, GS], f32)
    ident_d = din("ident", [128, 128], f32)
    out_d = nc.dram_tensor("out", [TOK, HID], f32, kind="ExternalOutput").ap()

    with tile.TileContext(nc) as tc:
        with tc.tile_pool(name="const", bufs=1) as cp, \
             tc.tile_pool(name="persist", bufs=1) as pp, \
             tc.tile_pool(name="ktp", bufs=3) as ktp, \
             tc.tile_pool(name="vp", bufs=3) as vpl:

            # K/V prefetch for the first batches — no deps, DMA can start
            # while projection weights stream in.
            kv_tiles = {}

            def load_kv(b):
                kt = ktp.tile([128, T], bf16, tag="kt")
                nc.scalar.dma_start(kt[:], kt_d[b])
                v = vpl.tile([128, NT, 132], bf16, tag="v")
                nc.sync.dma_start(v[:], v_d[b])
                kv_tiles[b] = (kt, v)

            for b in range(3):
                load_kv(b)

            ident = cp.tile([128, 128], f32, tag="ident")
            nc.sync.dma_start(ident[:], ident_d[:])
            cosq = cp.tile([128, 2, DH], f32, tag="cosq")
            nc.sync.dma_start(cosq[:], cosq_d[:])
            sinq = cp.tile([128, 2, DH], f32, tag="sinq")
            nc.sync.dma_start(sinq[:], sinq_d[:])
            cosk = cp.tile([128, 2, D], f32, tag="cosk")
            nc.sync.dma_start(cosk[:], cosk_d[:])
            sink = cp.tile([128, 2, D], f32, tag="sink")
            nc.sync.dma_start(sink[:], sink_d[:])
            maskt = cp.tile([S, GS], f32, tag="mask")
            nc.sync.dma_start(maskt[:], mask_d[:])

            QT = pp.tile([128, G, TOK], bf16, tag="QT")
            KTa = pp.tile([128, TOK], bf16, tag="KTa")
            VaT = pp.tile([128, TOK], f32, tag="VaT")
            attnT = pp.tile([128, G, TOK], bf16, tag="attnT")

            # ---------------- projections + RoPE ----------------
            with tc.tile_pool(name="projw", bufs=1) as pwl, \
                 tc.tile_pool(name="projsb", bufs=2) as psb, \
                 tc.tile_pool(name="projps", bufs=2, space="PSUM") as pps, \
                 tc.tile_pool(name="trps", bufs=2, space="PSUM") as tps:
                ht = pwl.tile([128, NH, TOK], bf16, tag="ht")
                wqs = pwl.tile([128, NH, DH], bf16, tag="wqs")
                wkvs = pwl.tile([128, NH, 2 * D], bf16, tag="wkvs")
                # half-loads so matmuls can start after the first half
                hm = NH // 2
                for h2 in range(2):
                    hs_ = slice(h2 * hm, (h2 + 1) * hm)
                    nc.sync.dma_start(ht[:, hs_, :], ht_d[:, hs_, :])
                    nc.sync.dma_start(wqs[:, hs_, :], wq_d[:, hs_, :])
                    nc.sync.dma_start(wkvs[:, hs_, :], wkv_d[:, hs_, :])

                for t2 in range(2):
                    ts_ = slice(t2 * 128, (t2 + 1) * 128)
                    # Q = hidden @ wq  -> [tok, (g, d)]
                    pq = pps.tile([128, DH], f32, tag="pq")
                    for hh in range(NH):
                        nc.tensor.matmul(pq[:], ht[:, hh, ts_], wqs[:, hh, :],
                                         start=(hh == 0), stop=(hh == NH - 1))
                    qsb = psb.tile([128, DH], f32, tag="qsb")
                    nc.vector.tensor_copy(qsb[:], pq[:])
                    rot = psb.tile([128, DH], f32, tag="rot")
                    for g in range(G):
                        o = g * D
                        nc.vector.tensor_copy(rot[:, o:o + 64], qsb[:, o + 64:o + 128])
                        nc.vector.tensor_copy(rot[:, o + 64:o + 128], qsb[:, o:o + 64])
                    t1 = psb.tile([128, DH], f32, tag="t1")
                    nc.vector.tensor_mul(t1[:], qsb[:], cosq[:, t2, :])
                    rot2 = psb.tile([128, DH], f32, tag="rot2")
                    nc.vector.tensor_mul(rot2[:], rot[:], sinq[:, t2, :])
                    qr = psb.tile([128, DH], f32, tag="qr")
                    nc.vector.tensor_add(qr[:], t1[:], rot2[:])
                    for g in range(G):
                        tp = tps.tile([128, 128], f32, tag="tp")
                        nc.tensor.transpose(tp[:], qr[:, g * D:(g + 1) * D], ident[:])
                        nc.vector.tensor_copy(QT[:, g, ts_], tp[:])

                    # K|V = hidden @ [wk | wv]  -> [tok, 2*d]
                    pkv = pps.tile([128, 2 * D], f32, tag="pkv")
                    for hh in range(NH):
                        nc.tensor.matmul(pkv[:], ht[:, hh, ts_], wkvs[:, hh, :],
                                         start=(hh == 0), stop=(hh == NH - 1))
                    ksb = psb.tile([128, D], f32, tag="ksb")
                    nc.vector.tensor_copy(ksb[:], pkv[:, 0:D])
                    rotk = psb.tile([128, D], f32, tag="rotk")
                    nc.vector.tensor_copy(rotk[:, 0:64], ksb[:, 64:128])
                    nc.vector.tensor_copy(rotk[:, 64:128], ksb[:, 0:64])
                    t1k = psb.tile([128, D], f32, tag="t1k")
                    nc.vector.tensor_mul(t1k[:], ksb[:], cosk[:, t2, :])
                    rotk2 = psb.tile([128, D], f32, tag="rotk2")
                    nc.vector.tensor_mul(rotk2[:], rotk[:], sink[:, t2, :])
                    kr = psb.tile([128, D], f32, tag="kr")
                    nc.vector.tensor_add(kr[:], t1k[:], rotk2[:])
                    tpk = tps.tile([128, 128], f32, tag="tp")
                    nc.tensor.transpose(tpk[:], kr[:], ident[:])
                    nc.vector.tensor_copy(KTa[:, ts_], tpk[:])
                    vsb = psb.tile([128, D], f32, tag="vsb")
                    nc.vector.tensor_copy(vsb[:], pkv[:, D:2 * D])
                    tpv = tps.tile([128, 128], f32, tag="tp")
                    nc.tensor.transpose(tpv[:], vsb[:], ident[:])
                    nc.vector.tensor_copy(VaT[:, ts_], tpv[:])

            # ---------------- attention + output projection ----------------
            with tc.tile_pool(name="wosb", bufs=1) as wop, \
                 tc.tile_pool(name="ep", bufs=3) as ep, \
                 tc.tile_pool(name="epa", bufs=2) as epa, \
                 tc.tile_pool(name="epi", bufs=2) as epi, \
                 tc.tile_pool(name="scps", bufs=2, space="PSUM") as scp, \
                 tc.tile_pool(name="acps", bufs=2, space="PSUM") as acp, \
                 tc.tile_pool(name="numps", bufs=2, space="PSUM") as nump, \
                 tc.tile_pool(name="wops", bufs=2, space="PSUM") as wops:
                wos = wop.tile([128, G, HID], bf16, tag="wos")
                nc.sync.dma_start(wos[:], wo_d[:])

                def emit_wo(t2):
                    ts_ = slice(t2 * 128, (t2 + 1) * 128)
                    osb = epi.tile([128, HID], f32, tag="osb")
                    for nch in range(8):
                        ns_ = slice(nch * 512, (nch + 1) * 512)
                        pw = wops.tile([128, 512], f32, tag="pw")
                        for g in range(G):
                            nc.tensor.matmul(pw[:], attnT[:, g, ts_],
                                             wos[:, g, ns_],
                                             start=(g == 0), stop=(g == G - 1))
                        nc.vector.tensor_copy(osb[:, ns_], pw[:])
                    nc.sync.dma_start(out_d[ts_, :], osb[:])

                for b in range(B):
                    kt, v = kv_tiles.pop(b)
                    if b + 3 < B:
                        load_kv(b + 3)

                    qb = QT[:, :, b * S:(b + 1) * S]
                    num = nump.tile([GS, D + 1], f32, tag="num")
                    for gi in range(NG):
                        sc = scp.tile([128, CG, GS], f32, tag="sc")
                        for ci in range(CG):
                            n = gi * CG + ci
                            nc.tensor.matmul(sc[:, ci, :],
                                             kt[:, n * 128:(n + 1) * 128], qb,
                                             start=True, stop=True)
                        e = ep.tile([128, CG, GS], bf16, tag="e")
                        nc.scalar.activation(e[:], sc[:], EXP, scale=SCALE)
                        for ci in range(CG):
                            n = gi * CG + ci
                            nc.tensor.matmul(num[:], e[:, ci, :], v[:, n, 0:D + 1],
                                             start=(n == 0), stop=False)
                    # speculative (causal-masked) segment
                    sca = acp.tile([S, GS], f32, tag="a")
                    nc.tensor.matmul(sca[:], KTa[:, b * S:(b + 1) * S], qb,
                                     start=True, stop=True)
                    nc.vector.tensor_add(sca[:], sca[:], maskt[:])
                    ea = epa.tile([S, GS], bf16, tag="ea")
                    nc.scalar.activation(ea[:], sca[:], EXP, scale=SCALE)
                    # bring this batch's active V rows back to partition base 0
                    tpv = acp.tile([S, D], f32, tag="a")
                    nc.tensor.transpose(tpv[:], VaT[:, b * S:(b + 1) * S], ident[:])
                    vb = epa.tile([S, D + 4], bf16, tag="vb")
                    nc.vector.tensor_copy(vb[:, 0:D], tpv[:])
                    nc.vector.memset(vb[:, D:D + 1], 1.0)
                    nc.tensor.matmul(num[:], ea[:], vb[:, 0:D + 1],
                                     start=False, stop=True)
                    # normalize + transpose into attnT
                    recip = epi.tile([GS, 1], f32, tag="recip")
                    nc.vector.reciprocal(recip[:], num[:, D:D + 1])
                    attn = epi.tile([GS, D], f32, tag="attn")
                    nc.vector.tensor_scalar_mul(attn[:], num[:, 0:D], recip[:])
                    tp2 = acp.tile([128, GS], f32, tag="a")
                    nc.tensor.transpose(tp2[:], attn[:], ident[0:GS, 0:GS])
                    nc.vector.tensor_copy(
                        attnT[:, :, b * S:(b + 1) * S],
                        tp2[:].rearrange("p (g s) -> p g s", g=G))
                    if b == 7:
                        emit_wo(0)
                emit_wo(1)

    nc.compile()
    return nc


def _pack(w, nchunks):
    # [nchunks*128, N] -> [128, nchunks, N]
    n = w.shape[1]
    return np.ascontiguousarray(
        w.reshape(nchunks, 128, n).transpose(1, 0, 2)).astype(ml_dtypes.bfloat16)


def _packf(w, nchunks):
    n = w.shape[1]
    return np.ascontiguousarray(
        w.reshape(nchunks, 128, n).transpose(1, 0, 2)).astype(np.float32)


def _vpad(vc):
    # [T, D] x B -> [B, 128, NT, 132] with a ones column at 128, zeros after
    v4 = vc.reshape(B, NT, 128, D).transpose(0, 2, 1, 3)
    out = np.zeros((B, 128, NT, 132), np.float32)
    out[..., :D] = v4
    out[..., D] = 1.0
    return out.astype(ml_dtypes.bfloat16)


def _prep(x):
    hs = np.asarray(x["hidden_states"], np.float32)
    wq = np.asarray(x["wq"], np.float32)
    wk = np.asarray(x["wk"], np.float32)
    wv = np.asarray(x["wv"], np.float32)
    wo = np.asarray(x["wo"], np.float32)
    kp = np.asarray(x["k_prior"], np.float32)
    vp = np.asarray(x["v_prior"], np.float32)

    ht_p = _pack(hs.reshape(TOK, HID).T, NH)

    pos = np.asarray(x["position_ids"]).astype(np.float32).reshape(-1)  # [TOK]
    inv = (1.0 / (10000.0 ** (np.arange(0, D, 2, dtype=np.float32)
                              / np.float32(D)))).astype(np.float32)
    ang = pos[:, None] * inv[None, :]
    emb = np.concatenate([ang, ang], axis=1)
    cos = np.cos(emb).astype(np.float32)
    sin = np.sin(emb).astype(np.float32)
    sin2 = np.concatenate([-sin[:, :64], sin[:, 64:]], axis=1).astype(np.float32)
    cosq = _packf(np.tile(cos, (1, G)), 2)
    sinq = _packf(np.tile(sin2, (1, G)), 2)
    cosk_p = _packf(cos, 2)
    sink_p = _packf(sin2, 2)

    am = np.asarray(x["active_mask"])[0, 0]  # [S, S] bool, row=query s, col=key t
    mask = np.where(am.T, np.float32(0.0), np.float32(-1e9)).astype(np.float32)
    maskf = np.ascontiguousarray(np.tile(mask, (1, G)))
    ident = np.eye(128, dtype=np.float32)

    maps = []
    for c in range(NCORES):
        maps.append(dict(
            ht=ht_p,
            wq=_pack(wq[:, c * DH:(c + 1) * DH], NH),
            wkv=_pack(np.concatenate([wk[:, c * D:(c + 1) * D],
                                      wv[:, c * D:(c + 1) * D]], axis=1), NH),
            wo=_pack(wo[c * DH:(c + 1) * DH, :], G),
            kt=np.ascontiguousarray(
                kp[:, c].transpose(0, 2, 1)).astype(ml_dtypes.bfloat16),
            v=_vpad(vp[:, c]),
            cosq=cosq, sinq=sinq, cosk=cosk_p, sink=sink_p,
            mask=maskf, ident=ident,
        ))
    return maps


def kernel(**inputs) -> np.ndarray:
    if "nc" not in _cache:
        _cache["nc"] = _build()
    nc = _cache["nc"]
    in_maps = _prep(inputs)
    res = run_bass_kernel_spmd(nc, in_maps, list(range(NCORES)),
                               **_cache.get("run_kwargs", {}))
    out = res.results[0]["out"].astype(np.float32).copy()
    for c in range(1, NCORES):
        out += res.results[c]["out"]
    if _cache.get("keep_result"):
        _cache["last_result"] = res
    return out.reshape(B, S, HID)


# revision 10
# speedup vs baseline: 3.0949x; 1.0879x over previous
"""Trainium2 Bass kernel for GQA decode-with-speculation attention.

Sharding: tensor-parallel over heads across 8 cores — core c owns kv head c
and query heads 4c..4c+3 (wq/wk/wv column-sharded, wo row-sharded, KV cache
sharded over kv heads). Each core computes a partial output [256, 4096]
(its heads' contribution through wo); the host sums the 8 partials.

Device-side strategy per core:
  - hiddenT [HID, TOK] fed from host, so projections need no on-device
    transpose of activations. Projections run in f32r (full PE rate at
    N>=256). RoPE is applied in token-major layout with sign-folded sin
    tables, then Q/K/V-active are PE-transposed to [d, tok].
  - K cache fed host-pre-transposed as [B, D, T] in bf16; V cache natural
    [B, T, D] in bf16. Attention matmuls run in bf16 (1 cyc/row vs 4 for
    fp32), accumulating in fp32 PSUM.
  - Scores computed transposed: scT[t, (g,s)] = KT_chunk.T @ QT_b into a
    shared [128, 512] PSUM tile (8 chunks per exp), exp on ScalarE
    straight out of PSUM (no max subtraction: |scaled scores| <~ 8, exp
    stays in fp32 range), then PV uses E^T as the stationary operand
    against [V | 1] so one accumulating matmul chain yields both the
    numerator [64, 128] and the softmax denominator (column 128) with the
    partition-dim reduction done by the PE.
  - Normalize with per-partition reciprocal broadcast, PE-transpose to
    attnT [d, (g, tok)], then wo row-chunks in f32r.
"""

import math
import sys

sys.path.insert(0, "/opt/trn_rl_repo")

import numpy as np
import ml_dtypes

import concourse.bass as bass
import concourse.tile as tile
from concourse import bacc, mybir
from concourse.bass_utils import run_bass_kernel_spmd

f32 = mybir.dt.float32
f32r = mybir.dt.float32r
bf16 = mybir.dt.bfloat16
EXP = mybir.ActivationFunctionType.Exp

B, S, T = 16, 16, 4096
H, HKV, D = 32, 8, 128
HID = H * D
G = H // HKV                 # 4 query heads per kv head
NCORES = 8
TOK = B * S                  # 256 tokens
DH = G * D                   # 512 per-core q/o dims
GS = G * S                   # 64 (g, s) rows per batch
NT = T // 128                # 32 prior key chunks
CG = 8                       # score chunks per exp group
NG = NT // CG                # 4 groups per batch
NH = HID // 128              # 32 hidden chunks
SCALE = 1.0 / math.sqrt(D)

_cache: dict = {}


def _build():
    nc = bacc.Bacc("TRN2", target_bir_lowering=False, debug=False,
                   num_devices=NCORES)

    def din(name, shape, dt):
        return nc.dram_tensor(name, shape, dt, kind="ExternalInput").ap()

    ht_d = din("ht", [128, NH, TOK], bf16)
    wq_d = din("wq", [128, NH, DH], bf16)
    wkv_d = din("wkv", [128, NH, 2 * D], bf16)
    wo_d = din("wo", [128, G, HID], bf16)
    kt_d = din("kt", [B, D, T], bf16)
    v_d = din("v", [B, 128, NT, 132], bf16)
    cosq_d = din("cosq", [128, 2, DH], f32)
    sinq_d = din("sinq", [128, 2, DH], f32)
    cosk_d = din("cosk", [128, 2, D], f32)
    sink_d = din("sink", [128, 2, D], f32)
    mask_d = din("mask", [S, GS], f32)
    ident_d = din("ident", [128, 128], f32)
    out_d = nc.dram_tensor("out", [TOK, HID], f32, kind="ExternalOutput").ap()

    with tile.TileContext(nc) as tc:
        with tc.tile_pool(name="const", bufs=1) as cp, \
             tc.tile_pool(name="persist", bufs=1) as pp, \
             tc.tile_pool(name="ktp", bufs=3) as ktp, \
             tc.tile_pool(name="vp", bufs=3) as vpl:

            # K/V prefetch for the first batches — kt on the ACT HWDGE ring,
            # v on the sync ring (after the projection weights).
            kv_tiles = {}

            def load_kv(b):
                kt = ktp.tile([128, T], bf16, tag="kt")
                nc.scalar.dma_start(kt[:], kt_d[b])
                v = vpl.tile([128, NT, 132], bf16, tag="v")
                nc.sync.dma_start(v[:], v_d[b])
                kv_tiles[b] = (kt, v)

            ident = cp.tile([128, 128], f32, tag="ident")
            nc.sync.dma_start(ident[:], ident_d[:])
            cosq = cp.tile([128, 2, DH], f32, tag="cosq")
            nc.sync.dma_start(cosq[:], cosq_d[:])
            sinq = cp.tile([128, 2, DH], f32, tag="sinq")
            nc.sync.dma_start(sinq[:], sinq_d[:])
            cosk = cp.tile([128, 2, D], f32, tag="cosk")
            nc.sync.dma_start(cosk[:], cosk_d[:])
            sink = cp.tile([128, 2, D], f32, tag="sink")
            nc.sync.dma_start(sink[:], sink_d[:])
            maskt = cp.tile([S, GS], f32, tag="mask")
            nc.sync.dma_start(maskt[:], mask_d[:])

            QT = pp.tile([128, G, TOK], bf16, tag="QT")
            KTa = pp.tile([128, TOK], bf16, tag="KTa")
            VaT = pp.tile([128, TOK], f32, tag="VaT")
            attnT = pp.tile([128, G, TOK], bf16, tag="attnT")

            # ---------------- projections + RoPE ----------------
            with tc.tile_pool(name="projw", bufs=1) as pwl, \
                 tc.tile_pool(name="projsb", bufs=2) as psb, \
                 tc.tile_pool(name="projps", bufs=2, space="PSUM") as pps, \
                 tc.tile_pool(name="trps", bufs=2, space="PSUM") as tps:
                ht = pwl.tile([128, NH, TOK], bf16, tag="ht")
                wqs = pwl.tile([128, NH, DH], bf16, tag="wqs")
                wkvs = pwl.tile([128, NH, 2 * D], bf16, tag="wkvs")
                # half-loads so matmuls can start after the first half;
                # interleave K/V prefetch so neither ring idles
                hm = NH // 2
                nc.sync.dma_start(ht[:, 0:hm, :], ht_d[:, 0:hm, :])
                nc.sync.dma_start(wqs[:, 0:hm, :], wq_d[:, 0:hm, :])
                load_kv(0)
                nc.sync.dma_start(ht[:, hm:NH, :], ht_d[:, hm:NH, :])
                nc.sync.dma_start(wqs[:, hm:NH, :], wq_d[:, hm:NH, :])
                nc.sync.dma_start(wkvs[:, 0:hm, :], wkv_d[:, 0:hm, :])
                nc.sync.dma_start(wkvs[:, hm:NH, :], wkv_d[:, hm:NH, :])
                load_kv(1)
                load_kv(2)

                for t2 in range(2):
                    ts_ = slice(t2 * 128, (t2 + 1) * 128)
                    # Q = hidden @ wq  -> [tok, (g, d)]
                    pq = pps.tile([128, DH], f32, tag="pq")
                    for hh in range(NH):
                        nc.tensor.matmul(pq[:], ht[:, hh, ts_], wqs[:, hh, :],
                                         start=(hh == 0), stop=(hh == NH - 1))
                    qsb = psb.tile([128, DH], f32, tag="qsb")
                    nc.vector.tensor_copy(qsb[:], pq[:])
                    rot = psb.tile([128, DH], f32, tag="rot")
                    for g in range(G):
                        o = g * D
                        nc.vector.tensor_copy(rot[:, o:o + 64], qsb[:, o + 64:o + 128])
                        nc.vector.tensor_copy(rot[:, o + 64:o + 128], qsb[:, o:o + 64])
                    t1 = psb.tile([128, DH], f32, tag="t1")
                    nc.vector.tensor_mul(t1[:], qsb[:], cosq[:, t2, :])
                    rot2 = psb.tile([128, DH], f32, tag="rot2")
                    nc.vector.tensor_mul(rot2[:], rot[:], sinq[:, t2, :])
                    qr = psb.tile([128, DH], f32, tag="qr")
                    nc.vector.tensor_add(qr[:], t1[:], rot2[:])
                    for g in range(G):
                        tp = tps.tile([128, 128], f32, tag="tp")
                        nc.tensor.transpose(tp[:], qr[:, g * D:(g + 1) * D], ident[:])
                        nc.vector.tensor_copy(QT[:, g, ts_], tp[:])

                    # K|V = hidden @ [wk | wv]  -> [tok, 2*d]
                    pkv = pps.tile([128, 2 * D], f32, tag="pkv")
                    for hh in range(NH):
                        nc.tensor.matmul(pkv[:], ht[:, hh, ts_], wkvs[:, hh, :],
                                         start=(hh == 0), stop=(hh == NH - 1))
                    ksb = psb.tile([128, D], f32, tag="ksb")
                    nc.vector.tensor_copy(ksb[:], pkv[:, 0:D])
                    rotk = psb.tile([128, D], f32, tag="rotk")
                    nc.vector.tensor_copy(rotk[:, 0:64], ksb[:, 64:128])
                    nc.vector.tensor_copy(rotk[:, 64:128], ksb[:, 0:64])
                    t1k = psb.tile([128, D], f32, tag="t1k")
                    nc.vector.tensor_mul(t1k[:], ksb[:], cosk[:, t2, :])
                    rotk2 = psb.tile([128, D], f32, tag="rotk2")
                    nc.vector.tensor_mul(rotk2[:], rotk[:], sink[:, t2, :])
                    kr = psb.tile([128, D], f32, tag="kr")
                    nc.vector.tensor_add(kr[:], t1k[:], rotk2[:])
                    tpk = tps.tile([128, 128], f32, tag="tp")
                    nc.tensor.transpose(tpk[:], kr[:], ident[:])
                    nc.vector.tensor_copy(KTa[:, ts_], tpk[:])
                    vsb = psb.tile([128, D], f32, tag="vsb")
                    nc.vector.tensor_copy(vsb[:], pkv[:, D:2 * D])
                    tpv = tps.tile([128, 128], f32, tag="tp")
                    nc.tensor.transpose(tpv[:], vsb[:], ident[:])
                    nc.vector.tensor_copy(VaT[:, ts_], tpv[:])

            # ---------------- attention + output projection ----------------
            with tc.tile_pool(name="wosb", bufs=1) as wop, \
                 tc.tile_pool(name="ep", bufs=4) as ep, \
                 tc.tile_pool(name="epa", bufs=2) as epa, \
                 tc.tile_pool(name="epi", bufs=2) as epi, \
                 tc.tile_pool(name="scps", bufs=2, space="PSUM") as scp, \
                 tc.tile_pool(name="acps", bufs=2, space="PSUM") as acp, \
                 tc.tile_pool(name="numps", bufs=2, space="PSUM") as nump, \
                 tc.tile_pool(name="wops", bufs=2, space="PSUM") as wops:
                wos = wop.tile([128, G, HID], bf16, tag="wos")
                nc.sync.dma_start(wos[:], wo_d[:])

                def emit_wo(t2):
                    ts_ = slice(t2 * 128, (t2 + 1) * 128)
                    osb = epi.tile([128, HID], f32, tag="osb")
                    for nch in range(8):
                        ns_ = slice(nch * 512, (nch + 1) * 512)
                        pw = wops.tile([128, 512], f32, tag="pw")
                        for g in range(G):
                            nc.tensor.matmul(pw[:], attnT[:, g, ts_],
                                             wos[:, g, ns_],
                                             start=(g == 0), stop=(g == G - 1))
                        nc.vector.tensor_copy(osb[:, ns_], pw[:])
                    nc.sync.dma_start(out_d[ts_, :], osb[:])

                for b in range(B):
                    kt, v = kv_tiles.pop(b)
                    if b + 3 < B:
                        load_kv(b + 3)

                    qb = QT[:, :, b * S:(b + 1) * S]
                    num = nump.tile([GS, D + 1], f32, tag="num")
                    for gi in range(NG):
                        sc = scp.tile([128, CG, GS], f32, tag="sc")
                        for ci in range(CG):
                            n = gi * CG + ci
                            nc.tensor.matmul(sc[:, ci, :],
                                             kt[:, n * 128:(n + 1) * 128], qb,
                                             start=True, stop=True)
                        e = ep.tile([128, CG, GS], bf16, tag="e")
                        nc.scalar.activation(e[:], sc[:], EXP, scale=SCALE)
                        for ci in range(CG):
                            n = gi * CG + ci
                            nc.tensor.matmul(num[:], e[:, ci, :], v[:, n, 0:D + 1],
                                             start=(n == 0), stop=False)
                    # speculative (causal-masked) segment
                    sca = acp.tile([S, GS], f32, tag="a")
                    nc.tensor.matmul(sca[:], KTa[:, b * S:(b + 1) * S], qb,
                                     start=True, stop=True)
                    nc.vector.tensor_add(sca[:], sca[:], maskt[:])
                    ea = epa.tile([S, GS], bf16, tag="ea")
                    nc.scalar.activation(ea[:], sca[:], EXP, scale=SCALE)
                    # bring this batch's active V rows back to partition base 0
                    tpv = acp.tile([S, D], f32, tag="a")
                    nc.tensor.transpose(tpv[:], VaT[:, b * S:(b + 1) * S], ident[:])
                    vb = epa.tile([S, D + 4], bf16, tag="vb")
                    nc.vector.tensor_copy(vb[:, 0:D], tpv[:])
                    nc.vector.memset(vb[:, D:D + 1], 1.0)
                    nc.tensor.matmul(num[:], ea[:], vb[:, 0:D + 1],
                                     start=False, stop=True)
                    # normalize + transpose into attnT
                    recip = epi.tile([GS, 1], f32, tag="recip")
                    nc.vector.reciprocal(recip[:], num[:, D:D + 1])
                    attn = epi.tile([GS, D], f32, tag="attn")
                    nc.vector.tensor_scalar_mul(attn[:], num[:, 0:D], recip[:])
                    tp2 = acp.tile([128, GS], f32, tag="a")
                    nc.tensor.transpose(tp2[:], attn[:], ident[0:GS, 0:GS])
                    nc.vector.tensor_copy(
                        attnT[:, :, b * S:(b + 1) * S],
                        tp2[:].rearrange("p (g s) -> p g s", g=G))
                    if b == 7:
                        emit_wo(0)
                emit_wo(1)

    nc.compile()
    return nc


def _pack(w, nchunks):
    # [nchunks*128, N] -> [128, nchunks, N]
    n = w.shape[1]
    return np.ascontiguousarray(
        w.reshape(nchunks, 128, n).transpose(1, 0, 2)).astype(ml_dtypes.bfloat16)


def _packf(w, nchunks):
    n = w.shape[1]
    return np.ascontiguousarray(
        w.reshape(nchunks, 128, n).transpose(1, 0, 2)).astype(np.float32)


def _vpad(vc):
    # [T, D] x B -> [B, 128, NT, 132] with a ones column at 128, zeros after
    v4 = vc.reshape(B, NT, 128, D).transpose(0, 2, 1, 3)
    out = np.zeros((B, 128, NT, 132), np.float32)
    out[..., :D] = v4
    out[..., D] = 1.0
    return out.astype(ml_dtypes.bfloat16)


def _prep(x):
    hs = np.asarray(x["hidden_states"], np.float32)
    wq = np.asarray(x["wq"], np.float32)
    wk = np.asarray(x["wk"], np.float32)
    wv = np.asarray(x["wv"], np.float32)
    wo = np.asarray(x["wo"], np.float32)
    kp = np.asarray(x["k_prior"], np.float32)
    vp = np.asarray(x["v_prior"], np.float32)

    ht_p = _pack(hs.reshape(TOK, HID).T, NH)

    pos = np.asarray(x["position_ids"]).astype(np.float32).reshape(-1)  # [TOK]
    inv = (1.0 / (10000.0 ** (np.arange(0, D, 2, dtype=np.float32)
                              / np.float32(D)))).astype(np.float32)
    ang = pos[:, None] * inv[None, :]
    emb = np.concatenate([ang, ang], axis=1)
    cos = np.cos(emb).astype(np.float32)
    sin = np.sin(emb).astype(np.float32)
    sin2 = np.concatenate([-sin[:, :64], sin[:, 64:]], axis=1).astype(np.float32)
    cosq = _packf(np.tile(cos, (1, G)), 2)
    sinq = _packf(np.tile(sin2, (1, G)), 2)
    cosk_p = _packf(cos, 2)
    sink_p = _packf(sin2, 2)

    am = np.asarray(x["active_mask"])[0, 0]  # [S, S] bool, row=query s, col=key t
    mask = np.where(am.T, np.float32(0.0), np.float32(-1e9)).astype(np.float32)
    maskf = np.ascontiguousarray(np.tile(mask, (1, G)))
    ident = np.eye(128, dtype=np.float32)

    maps = []
    for c in range(NCORES):
        maps.append(dict(
            ht=ht_p,
            wq=_pack(wq[:, c * DH:(c + 1) * DH], NH),
            wkv=_pack(np.concatenate([wk[:, c * D:(c + 1) * D],
                                      wv[:, c * D:(c + 1) * D]], axis=1), NH),
            wo=_pack(wo[c * DH:(c + 1) * DH, :], G),
            kt=np.ascontiguousarray(
                kp[:, c].transpose(0, 2, 1)).astype(ml_dtypes.bfloat16),
            v=_vpad(vp[:, c]),
            cosq=cosq, sinq=sinq, cosk=cosk_p, sink=sink_p,
            mask=maskf, ident=ident,
        ))
    return maps


def kernel(**inputs) -> np.ndarray:
    if "nc" not in _cache:
        _cache["nc"] = _build()
    nc = _cache["nc"]
    in_maps = _prep(inputs)
    res = run_bass_kernel_spmd(nc, in_maps, list(range(NCORES)),
                               **_cache.get("run_kwargs", {}))
    out = res.results[0]["out"].astype(np.float32).copy()
    for c in range(1, NCORES):
        out += res.results[c]["out"]
    if _cache.get("keep_result"):
        _cache["last_result"] = res
    return out.reshape(B, S, HID)
